# revision 1
# baseline (speedup 1.0000x reference)
"""MQA self-attention kernel for Trainium2, 8 NeuronCores.

Reference computation (fp32):
    q = x @ wq.T + bq        -> [B,S,1024] -> heads via (hidden num_heads) split
    k = x @ wk.T + bk        -> [B,S,64]  (single shared KV head)
    v = x @ wv.T + bv
    scores = q @ k.T / 8 ; attn = softmax(scores) ; h = attn @ v
    out = merge_heads(h) @ wo.T + bo

Sharding (8 cores, no collectives): core c handles batch b=c//4 and head
group g=c%4 (4 of the 16 q-heads).  The shared K/V head is replicated.
Each core returns the partial output h_g @ wo_g.T [S, D]; the host sums
the 4 head-group partials per batch and adds the bias terms.

Math notes:
 - bk provably cancels in softmax (adds a per-row constant to scores).
 - bv is folded into the output bias on host: softmax rows sum to 1, so
   attn @ (v + bv) = attn @ v + bv, contributing wo @ tile(bv, 16).
 - softmax is computed without max subtraction (scores ~ N(0,1); exp is
   safe in fp32) which lets exp(scores) @ [V|1] accumulate both the
   numerator and denominator in one PSUM pass.

Device layout (everything transposed so no on-device input transposes):
 - xT [1024, 2048] (d on partitions) is the rhs for all projections.
 - QT[q,s] and KT[d,s] computed directly in transposed layout; both are
   duplicated across SBUF partition halves so the scores matmul (K=64)
   runs as 64x128 row-tiled pairs over two key tiles at once.
 - scoresT[sk,sq] = KT.T @ QT per head; exp on ScalarE in [128,1024]
   blocks (amortizes the per-ACTIVATE overhead), PV matmul with V'=[V|1]
   (M=65) gives hT_un[d,sq] plus the softmax denominator in the same
   PSUM accumulation.
 - normalize via DVE reciprocal + broadcast-DMA + DVE multiply.
 - out partial = hT.T @ woT via PSUM accumulation over the 256 q dims.
All matmuls run in float32r (TF32-like, full PE rate at N=512).
"""

import numpy as np

NUM_HEADS = 16
Dh = 64
B, S, D = 2, 2048, 1024
G = 4            # head groups (cores per batch)
HG = 4           # heads per group
QD = HG * Dh     # 256 local q dims
NK = D // 128    # 8 contraction tiles for projections
NSK = S // 128   # 16 key tiles
W = 512          # matmul moving width
BLK = 1024       # sq block width for exp
NB = S // BLK    # 2 blocks
N_CORES = 8

_CACHE = {}


def _build_nc():
    from contextlib import ExitStack

    import concourse.bass as bass
    import concourse.mybir as mybir
    import concourse.tile as tile
    from concourse import bacc
    from concourse.masks import make_identity

    F32 = mybir.dt.float32
    F32R = mybir.dt.float32r
    EXP = mybir.ActivationFunctionType.Exp

    nc = bacc.Bacc("TRN2", target_bir_lowering=False, debug=False)

    xT = nc.declare_dram_parameter("xT", [D, S], F32R, isOutput=False)
    wqT = nc.declare_dram_parameter("wqT", [D, QD], F32R, isOutput=False)
    wvkT = nc.declare_dram_parameter("wvkT", [D, 128], F32R, isOutput=False)
    woT = nc.declare_dram_parameter("woT", [QD, D], F32R, isOutput=False)
    bqp = nc.declare_dram_parameter("bq", [QD, 1], F32, isOutput=False)
    part = nc.declare_dram_parameter("part", [S, D], F32, isOutput=True)

    with tile.TileContext(nc) as tc, ExitStack() as ctx:
        const = ctx.enter_context(tc.tile_pool(name="const", bufs=1))
        persist = ctx.enter_context(tc.tile_pool(name="persist", bufs=1))

        wq_sb = const.tile([128, NK * QD], F32R)    # ktile kt at cols [kt*QD:+QD]
        wvk_sb = const.tile([128, NK * 128], F32R)  # cols 0:64 of each ktile = wvT, 64:128 = wkT
        wo_sb = const.tile([128, 2 * D], F32R)      # q-ktile p at cols [p*D:+D]
        bq_sb = const.tile([128, 2], F32)
        ident = const.tile([128, 128], F32)
        ones_sb = const.tile([128, 1], F32)

        # qtd: per-head QT duplicated across both partition halves:
        # head h at cols [h*S:+S], rows 0:64 == rows 64:128 == QT_h [64, S]
        qtd_sb = persist.tile([128, HG * S], F32R)
        qod_sb = persist.tile([128, 2 * S], F32R)   # odd heads' QT at rows 0:64
        ktd_sb = persist.tile([128, S], F32R)       # KT staging (rows 64:128)
        kt0_sb = persist.tile([128, S], F32R)       # KT at rows 0:64
        v1_sb = persist.tile([128, NSK * 65], F32R)  # V' tile sk at cols [sk*65:+65]
        ht_sb = persist.tile([128, 2 * S], F32R)    # normalized hT, q-ktile p at cols [p*S:+S]

        make_identity(nc, ident[:])
        nc.vector.memset(ones_sb[:], 1.0)

        # ---- Phase 1: projections -------------------------------------
        with tc.tile_pool(name="xp", bufs=1) as xp:
            x_sb = xp.tile([128, NK * S], F32R)     # xT ktile kt at cols [kt*S:+S]
            vt_sb = xp.tile([128, S], F32)          # VT in rows 0:64
            for kt in range(NK):
                nc.sync.dma_start(wvk_sb[:, kt * 128:(kt + 1) * 128], wvkT[kt * 128:(kt + 1) * 128, :])
            for kt in range(NK):
                nc.sync.dma_start(wq_sb[:, kt * QD:(kt + 1) * QD], wqT[kt * 128:(kt + 1) * 128, :])
            for kt in range(NK):
                nc.sync.dma_start(x_sb[:, kt * S:(kt + 1) * S], xT[kt * 128:(kt + 1) * 128, :])
            for p in range(2):
                nc.sync.dma_start(bq_sb[:, p:p + 1], bqp[p * 128:(p + 1) * 128, :])
                nc.sync.dma_start(wo_sb[:, p * D:(p + 1) * D], woT[p * 128:(p + 1) * 128, :])

            # fused [V|K] projection interleaved with QT heads 0/1 so both
            # finish right as the last x tile lands.
            with (
                tc.tile_pool(name="vkps", bufs=1, space="PSUM") as vkps,
                tc.tile_pool(name="qps", bufs=1, space="PSUM") as qps,
            ):
                vk_ps = vkps.tile([128, S], F32)
                q_ps = qps.tile([128, S], F32)
                for kt in range(NK):
                    for n in range(S // W):
                        nc.tensor.matmul(
                            vk_ps[:, n * W:(n + 1) * W],
                            lhsT=wvk_sb[:, kt * 128:(kt + 1) * 128],
                            rhs=x_sb[:, kt * S + n * W: kt * S + (n + 1) * W],
                            start=(kt == 0), stop=(kt == NK - 1),
                        )
                    for n in range(S // W):
                        nc.tensor.matmul(
                            q_ps[:, n * W:(n + 1) * W],
                            lhsT=wq_sb[:, kt * QD: kt * QD + 128],
                            rhs=x_sb[:, kt * S + n * W: kt * S + (n + 1) * W],
                            start=(kt == 0), stop=(kt == NK - 1),
                        )
                # evacs split across DVE and the (still idle) ScalarE so the
                # path to the first scores matmul is short
                nc.vector.tensor_scalar_add(
                    qtd_sb[0:64, 0:S], q_ps[0:64, :], bq_sb[0:64, 0:1]
                )
                nc.scalar.copy(ktd_sb[64:128, :], vk_ps[64:128, :])
                nc.gpsimd.tensor_copy(kt0_sb[0:64, :], ktd_sb[64:128, :])
                nc.scalar.copy(vt_sb[0:64, :], vk_ps[0:64, :])
                nc.vector.tensor_scalar_add(
                    qtd_sb[64:128, S:2 * S], q_ps[64:128, :], bq_sb[64:128, 0:1]
                )
                nc.gpsimd.tensor_copy(qod_sb[0:64, 0:S], qtd_sb[64:128, S:2 * S])

            # QT heads 2/3 (reuses the freed PSUM banks)
            with tc.tile_pool(name="qps2", bufs=1, space="PSUM") as qps2:
                q_ps2 = qps2.tile([128, S], F32)
                for kt in range(NK):
                    for n in range(S // W):
                        nc.tensor.matmul(
                            q_ps2[:, n * W:(n + 1) * W],
                            lhsT=wq_sb[:, kt * QD + 128: kt * QD + 256],
                            rhs=x_sb[:, kt * S + n * W: kt * S + (n + 1) * W],
                            start=(kt == 0), stop=(kt == NK - 1),
                        )
                nc.vector.tensor_scalar_add(
                    qtd_sb[0:64, 2 * S:3 * S], q_ps2[0:64, :], bq_sb[0:64, 1:2]
                )
                nc.vector.tensor_scalar_add(
                    qtd_sb[64:128, 3 * S:4 * S], q_ps2[64:128, :], bq_sb[64:128, 1:2]
                )
                nc.gpsimd.tensor_copy(qod_sb[0:64, S:2 * S], qtd_sb[64:128, 3 * S:4 * S])

            # V' tiles: PE-transpose VT -> V[sk] = [128, 64], plus ones column
            with tc.tile_pool(name="trps", bufs=2, space="PSUM") as trps:
                for sk in range(NSK):
                    tr_ps = trps.tile([128, Dh], F32)
                    nc.tensor.transpose(
                        tr_ps[:], vt_sb[0:64, sk * 128:(sk + 1) * 128], ident[0:64, 0:64]
                    )
                    nc.vector.tensor_copy(v1_sb[:, sk * 65: sk * 65 + 64], tr_ps[:])
                    nc.vector.tensor_copy(v1_sb[:, sk * 65 + 64: sk * 65 + 65], ones_sb[:])

        # ---- Phase 2/3: attention + output projection ------------------
        # Software-pipelined at key-tile granularity: scores(sk) -> exp(sk)
        # -> PV(sk-2), so ScalarE (the exp bottleneck) never idles.
        with (
            tc.tile_pool(name="expp", bufs=6) as expp,
            tc.tile_pool(name="scps", bufs=2, space="PSUM") as scps,
            tc.tile_pool(name="pvps", bufs=1, space="PSUM") as pvps,
            tc.tile_pool(name="outps", bufs=2, space="PSUM") as outps,
            tc.tile_pool(name="smalls", bufs=4) as smalls,
            tc.tile_pool(name="bcp", bufs=4) as bcp,
            tc.tile_pool(name="osbp", bufs=3) as osbp,
        ):
            for b in range(NB):
                for h in range(HG):
                    qcol = (h if h % 2 == 0 else h // 2) * S + b * BLK
                    hv = [pvps.tile([128, W], F32, name=f"pv{half}")
                          for half in range(BLK // W)]
                    exp_tiles = [None] * NSK

                    def emit_pv(sk):
                        for half in range(BLK // W):
                            nc.tensor.matmul(
                                hv[half][0:65, :],
                                lhsT=v1_sb[:, sk * 65:(sk + 1) * 65],
                                rhs=exp_tiles[sk][:, half * W:(half + 1) * W],
                                start=(sk == 0), stop=(sk == NSK - 1),
                            )

                    for sk in range(NSK):
                        if sk >= 2:
                            emit_pv(sk - 2)
                        sc = scps.tile([128, BLK], F32, name="sc")
                        qsrc = qtd_sb if h % 2 == 0 else qod_sb
                        for n in range(BLK // W):
                            nc.tensor.matmul(
                                sc[:, n * W:(n + 1) * W],
                                lhsT=kt0_sb[0:64, sk * 128:(sk + 1) * 128],
                                rhs=qsrc[0:64, qcol + n * W: qcol + (n + 1) * W],
                                start=True, stop=True,
                            )
                        et = expp.tile([128, BLK], F32R, name="expt")
                        nc.scalar.activation(et[:], sc[:], EXP, scale=0.125)
                        exp_tiles[sk] = et
                    emit_pv(NSK - 2)
                    emit_pv(NSK - 1)

                    # normalize: hT[:, sq] /= sumexp[sq]
                    for half in range(BLK // W):
                        rec = smalls.tile([128, W], F32, name="rec")
                        nc.vector.reciprocal(rec[64:65, :], hv[half][64:65, :])
                        rec0 = smalls.tile([128, W], F32, name="rec0")
                        nc.gpsimd.tensor_copy(rec0[0:1, :], rec[64:65, :])
                        bc = bcp.tile([128, W], F32, name="bc")
                        nc.gpsimd.partition_broadcast(bc[0:64, :], rec0[0:1, :], channels=64)
                        hcol = (h // 2) * S + b * BLK + half * W
                        if h % 2 == 0:
                            nc.vector.tensor_mul(
                                ht_sb[0:64, hcol:hcol + W], hv[half][0:64, :], bc[0:64, :]
                            )
                        else:
                            tmp = bcp.tile([128, W], F32R, name="tmp")
                            nc.vector.tensor_mul(tmp[0:64, :], hv[half][0:64, :], bc[0:64, :])
                            # odd head lives in ht rows 64:128 (GpSimd partition shift)
                            nc.gpsimd.tensor_copy(ht_sb[64:128, hcol:hcol + W], tmp[0:64, :])

                # output projection for the 8 s-chunks of this block
                for sc_i in range(BLK // 128):
                    s = b * (BLK // 128) + sc_i
                    for n in range(2):
                        o_ps = outps.tile([128, W], F32, name="ops")
                        for p in range(2):
                            nc.tensor.matmul(
                                o_ps[:],
                                lhsT=ht_sb[:, p * S + s * 128: p * S + (s + 1) * 128],
                                rhs=wo_sb[:, p * D + n * W: p * D + (n + 1) * W],
                                start=(p == 0), stop=(p == 1),
                            )
                        o_sb = osbp.tile([128, W], F32, name="osb")
                        nc.vector.tensor_copy(o_sb[:], o_ps[:])
                        nc.sync.dma_start(
                            part[s * 128:(s + 1) * 128, n * W:(n + 1) * W], o_sb[:]
                        )

    nc.finalize()
    return nc


def _get_nc():
    if "nc" not in _CACHE:
        _CACHE["nc"] = _build_nc()
    return _CACHE["nc"]


def _prep_core_inputs(inputs, wq, bq, wk, wv, wo):
    """Host-side shard prep: per-core transposed/rearranged operands."""
    xT = [np.ascontiguousarray(np.asarray(inputs[b], np.float32).T) for b in range(B)]
    wq3 = np.asarray(wq, np.float32).reshape(Dh, NUM_HEADS, D)
    bq2 = np.asarray(bq, np.float32).reshape(Dh, NUM_HEADS)
    wvkT = np.ascontiguousarray(
        np.concatenate([np.asarray(wv, np.float32).T, np.asarray(wk, np.float32).T], axis=1)
    )  # [1024, 128]
    wo_ = np.asarray(wo, np.float32)

    in_maps = []
    for c in range(N_CORES):
        b, g = divmod(c, G)
        heads = [g * HG + hl for hl in range(HG)]
        # wqT_g [1024, 256]: column block hl = head (g*HG+hl), rows = d
        wqT_g = np.ascontiguousarray(
            np.concatenate([wq3[:, h, :].T for h in heads], axis=1)
        )
        bq_g = np.ascontiguousarray(
            np.concatenate([bq2[:, h] for h in heads]).reshape(QD, 1)
        )
        woT_g = np.ascontiguousarray(wo_[:, g * QD:(g + 1) * QD].T)  # [256, 1024]
        in_maps.append({
            "xT": xT[b],
            "wqT": wqT_g,
            "wvkT": wvkT,
            "woT": woT_g,
            "bq": bq_g,
        })
    return in_maps


def kernel(inputs, wq, bq, wk, bk, wv, bv, wo, bo):
    from concourse.bass_utils import run_bass_kernel_spmd

    nc = _get_nc()
    in_maps = _prep_core_inputs(inputs, wq, bq, wk, wv, wo)
    res = run_bass_kernel_spmd(nc, in_maps, list(range(N_CORES))).results

    wo_ = np.asarray(wo, np.float32)
    bias = (
        np.asarray(bo, np.float32)
        + wo_ @ np.tile(np.asarray(bv, np.float32), NUM_HEADS)
    )
    out = np.empty((B, S, D), np.float32)
    for b in range(B):
        acc = res[b * G]["part"].astype(np.float32).copy()
        for g in range(1, G):
            acc += res[b * G + g]["part"]
        out[b] = acc + bias
    return out



# revision 19
# speedup vs baseline: 1.1200x; 1.1200x over previous
"""MQA self-attention kernel for Trainium2, 8 NeuronCores.

Reference computation (fp32):
    q = x @ wq.T + bq        -> [B,S,1024] -> heads via (hidden num_heads) split
    k = x @ wk.T + bk        -> [B,S,64]  (single shared KV head)
    v = x @ wv.T + bv
    scores = q @ k.T / 8 ; attn = softmax(scores) ; h = attn @ v
    out = merge_heads(h) @ wo.T + bo

Sharding (8 cores, no collectives): core c handles batch b=c//4 and head
group g=c%4 (4 of the 16 q-heads).  The shared K/V head is replicated.
Each core returns the partial output h_g @ wo_g.T [S, D]; the host sums
the 4 head-group partials per batch and adds the bias terms.

Math notes:
 - bk provably cancels in softmax; bv is folded into the host-side output
   bias (softmax rows sum to 1); softmax runs without max subtraction
   (scores ~ N(0,1), exp stays within bf16/f32 range).

Device pipeline (all operands bf16, PSUM f32; the Activation engine's
exp throughput ~133us/core is the hard floor, so every other engine is
kept strictly below it):
 - xT/weights land as bf16 (halves the input DMA), projections produce
   QT [2 heads stacked per 128 partitions], KT (two partition-offset
   copies so odd/even heads both get offset-matched operands) and VT.
 - V' = [V|1] built by PE-transpose (half 0) / DMA-transpose (half 1).
 - scores_T[k, q] per head per 1024-query block; exp on ScalarE in
   [128,1024] blocks writing bf16.
 - PV runs dense: h_un[q, 65] += exp_T[:, qtile].T @ V' accumulated over
   key tiles in PSUM (2x fewer PE rows than the hT-layout alternative).
 - normalize on eviction via per-partition reciprocal multiply; pairs of
   heads share an SBUF buffer that one DMA-transpose per pair flips into
   hT layout for the output projection.
 - half-1 projections and out-projection chunks are emitted as deferred
   PE tasks, one per exp slot, so the Tensor engine queue always has
   work but never starves the exp pipeline.
"""

from collections import deque

import numpy as np

NUM_HEADS = 16
Dh = 64
B, S, D = 2, 2048, 1024
G = 4            # head groups (cores per batch)
HG = 4           # heads per group
QD = HG * Dh     # 256 local q dims
NK = D // 128    # 8 contraction tiles for projections
NSK = S // 128   # 16 key tiles
W = 512          # matmul moving width
HALF = 1024      # query block / projection column half
N_CORES = 8

_CACHE = {}
_DEBUG = False


def _build_nc():
    from contextlib import ExitStack

    import concourse.bass as bass
    import concourse.mybir as mybir
    import concourse.tile as tile
    from concourse import bacc
    from concourse.masks import make_identity

    F32 = mybir.dt.float32
    BF16 = mybir.dt.bfloat16
    EXP = mybir.ActivationFunctionType.Exp

    nc = bacc.Bacc("TRN2", target_bir_lowering=False, debug=False)

    xT = nc.declare_dram_parameter("xT", [D, S], BF16, isOutput=False)
    wqT = nc.declare_dram_parameter("wqT", [D, QD], BF16, isOutput=False)
    wkvT = nc.declare_dram_parameter("wkvT", [D, 128], BF16, isOutput=False)
    woT = nc.declare_dram_parameter("woT", [QD, D], BF16, isOutput=False)
    bqp = nc.declare_dram_parameter("bq", [QD, 1], F32, isOutput=False)
    part = nc.declare_dram_parameter("part", [S, D], F32, isOutput=True)
    if _DEBUG:
        dbg = {
            "d_qt": nc.declare_dram_parameter("d_qt", [128, 2 * S], BF16, isOutput=True),
            "d_kt2": nc.declare_dram_parameter("d_kt2", [128, S], BF16, isOutput=True),
            "d_v1": nc.declare_dram_parameter("d_v1", [128, NSK * 65], BF16, isOutput=True),
            "d_ht": nc.declare_dram_parameter("d_ht", [128, 2 * S], BF16, isOutput=True),
        }

    with tile.TileContext(nc) as tc, ExitStack() as ctx:
        const = ctx.enter_context(tc.tile_pool(name="const", bufs=1))
        persist = ctx.enter_context(tc.tile_pool(name="persist", bufs=1))

        wq_sb = const.tile([128, NK * QD], BF16)    # ktile kt at cols [kt*QD:+QD]
        wkv_sb = const.tile([128, NK * 128], BF16)  # cols 0:64 = wkT, 64:128 = wvT
        wo_sb = const.tile([128, 2 * D], BF16)      # q-ktile p at cols [p*D:+D]
        bq_sb = const.tile([128, 2], F32)
        ident = const.tile([128, 128], BF16)

        qt_sb = persist.tile([128, 2 * S], BF16)    # pt p cols [p*S:+S]; rows 0:64 head 2p, 64:128 head 2p+1
        kt2_sb = persist.tile([128, S], BF16)       # KT duplicated rows 0:64 and 64:128
        vt_sb = persist.tile([128, S], BF16)        # VT in rows 64:128
        v1_sb = persist.tile([128, NSK * 65], BF16)  # V' tile sk at cols [sk*65:+65]
        ht_sb = persist.tile([128, 2 * S], BF16)    # hT, q-ktile p at cols [p*S:+S]

        make_identity(nc, ident[:])
        nc.vector.memset(v1_sb[:], 1.0)  # pre-fill the softmax-denominator columns

        # ---- DMAs + half-0 projections --------------------------------
        xp = ctx.enter_context(tc.tile_pool(name="xp", bufs=1))
        xt = [[xp.tile([128, HALF], BF16, name=f"x{kt}_{hf}") for hf in range(2)]
              for kt in range(NK)]

        nc.sync.dma_start(
            wkv_sb[:].rearrange("p (k c) -> p k c", c=128),
            wkvT[:, :].rearrange("(k p) c -> p k c", p=128),
        )
        nc.sync.dma_start(
            wq_sb[:].rearrange("p (k c) -> p k c", c=QD),
            wqT[:, :].rearrange("(k p) c -> p k c", p=128),
        )
        for p in range(2):
            nc.sync.dma_start(bq_sb[:, p:p + 1], bqp[p * 128:(p + 1) * 128, :])
        nc.sync.dma_start(
            wo_sb[:].rearrange("p (a c) -> p a c", c=D),
            woT[:, :].rearrange("(a p) c -> p a c", p=128),
        )
        for hf in range(2):
            for kt in range(NK):
                nc.sync.dma_start(
                    xt[kt][hf][:],
                    xT[kt * 128:(kt + 1) * 128, hf * HALF:(hf + 1) * HALF],
                )

        def proj_kt_step(ps, wsb_col, kt, hf, width):
            for n in range(HALF // W):
                nc.tensor.matmul(
                    ps[:, n * W:(n + 1) * W],
                    lhsT=wsb_col(kt),
                    rhs=xt[kt][hf][:, n * W:(n + 1) * W],
                    start=(kt == 0), stop=(kt == NK - 1),
                )

        wkv_col = lambda kt: wkv_sb[:, kt * 128:(kt + 1) * 128]
        wq0_col = lambda kt: wq_sb[:, kt * QD:kt * QD + 128]
        wq1_col = lambda kt: wq_sb[:, kt * QD + 128:kt * QD + 256]

        with (
            tc.tile_pool(name="p0", bufs=1, space="PSUM") as p0,
            tc.tile_pool(name="trps", bufs=2, space="PSUM") as trps,
        ):
            vk_ps = p0.tile([128, HALF], F32, name="vk")
            q0_ps = p0.tile([128, HALF], F32, name="q0")
            q1_ps = p0.tile([128, HALF], F32, name="q1")
            for kt in range(NK):
                proj_kt_step(vk_ps, wkv_col, kt, 0, HALF)
                proj_kt_step(q0_ps, wq0_col, kt, 0, HALF)
                proj_kt_step(q1_ps, wq1_col, kt, 0, HALF)
            # evictions: K first (scores need it first); ACT is idle here.
            nc.scalar.copy(kt2_sb[0:64, 0:HALF], vk_ps[0:64, :])
            nc.scalar.copy(vt_sb[64:128, 0:HALF], vk_ps[64:128, :])
            nc.gpsimd.tensor_copy(kt2_sb[64:128, 0:HALF], kt2_sb[0:64, 0:HALF])
            nc.vector.tensor_scalar_add(qt_sb[:, 0:HALF], q0_ps[:], bq_sb[:, 0:1])
            nc.vector.tensor_scalar_add(qt_sb[:, S:S + HALF], q1_ps[:], bq_sb[:, 1:2])
            # V' half 0 by PE transpose (DMA engines are busy with x half 1)
            for sk in range(NSK // 2):
                tr = trps.tile([128, Dh], BF16, name="tr")
                nc.tensor.transpose(
                    tr[:], vt_sb[64:128, sk * 128:(sk + 1) * 128],
                    ident[64:128, 64:128],
                )
                nc.vector.tensor_copy(v1_sb[:, sk * 65:sk * 65 + 64], tr[:])

        # ---- Phase 2: attention, with deferred fill-in PE tasks -------
        scps = ctx.enter_context(tc.tile_pool(name="scps", bufs=2, space="PSUM"))
        hups = ctx.enter_context(tc.tile_pool(name="hups", bufs=1, space="PSUM"))
        expp = ctx.enter_context(tc.tile_pool(name="expp", bufs=5))
        hpp = ctx.enter_context(tc.tile_pool(name="hpp", bufs=2))
        smalls = ctx.enter_context(tc.tile_pool(name="smalls", bufs=4))
        osbp = ctx.enter_context(tc.tile_pool(name="osbp", bufs=2))

        state = {}

        def t_vk1_a():
            pvk = pvk_stack.enter_context(tc.tile_pool(name="pvk", bufs=1, space="PSUM"))
            state["vk1"] = pvk.tile([128, HALF], F32, name="vkps")
            for kt in range(4):
                proj_kt_step(state["vk1"], wkv_col, kt, 1, HALF)

        def t_vk1_b():
            for kt in range(4, NK):
                proj_kt_step(state["vk1"], wkv_col, kt, 1, HALF)

        def t_kv1_evict():
            vk1 = state.pop("vk1")
            nc.vector.tensor_copy(kt2_sb[0:64, HALF:S], vk1[0:64, :])
            nc.vector.tensor_copy(vt_sb[64:128, HALF:S], vk1[64:128, :])
            nc.gpsimd.tensor_copy(kt2_sb[64:128, HALF:S], kt2_sb[0:64, HALF:S])
            pvk_stack.close()

        def t_v1_h1():
            # V' half 1 by PE transpose, in a short-lived 1-bank PSUM era
            # (dma_start_transpose mis-executes the 65-strided pattern on HW)
            with tc.tile_pool(name="trps1", bufs=2, space="PSUM") as trps1:
                for sk in range(NSK // 2, NSK):
                    tr = trps1.tile([128, Dh], BF16, name="tr1")
                    nc.tensor.transpose(
                        tr[:], vt_sb[64:128, sk * 128:(sk + 1) * 128],
                        ident[64:128, 64:128],
                    )
                    nc.vector.tensor_copy(v1_sb[:, sk * 65:sk * 65 + 64], tr[:])

        def mk_q1(which, col_fn, kts):
            def t():
                if which in state:
                    ps = state[which]
                else:
                    if "pq" not in state:
                        state["pq"] = pq_stack.enter_context(
                            tc.tile_pool(name="pq", bufs=1, space="PSUM"))
                    ps = state[which] = state["pq"].tile([128, HALF], F32, name="qps")
                for kt in kts:
                    proj_kt_step(ps, col_fn, kt, 1, HALF)
            return t

        def mk_q1_evict(which, pt):
            def t():
                ps = state.pop(which)
                nc.vector.tensor_scalar_add(
                    qt_sb[:, pt * S + HALF:pt * S + S], ps[:], bq_sb[:, pt:pt + 1]
                )
            return t

        def t_close_p1b():
            pq_stack.close()

        tasks = deque([
            t_vk1_a, t_vk1_b, t_kv1_evict, t_v1_h1,
            mk_q1("q0h1", wq0_col, range(4)), mk_q1("q0h1", wq0_col, range(4, NK)),
            mk_q1_evict("q0h1", 0),
            mk_q1("q1h1", wq1_col, range(4)), mk_q1("q1h1", wq1_col, range(4, NK)),
            mk_q1_evict("q1h1", 1),
            t_close_p1b,
        ])

        outps_stack = ExitStack()
        outps = None
        pvk_stack = ExitStack()
        pq_stack = ExitStack()

        def mk_outproj(bI, sc_i):
            def t():
                nonlocal outps
                if outps is None:
                    outps = outps_stack.enter_context(
                        tc.tile_pool(name="outps", bufs=2, space="PSUM"))
                s = bI * (HALF // 128) + sc_i
                o_sb = osbp.tile([128, D], F32, name="osb")
                for n in range(2):
                    o_ps = outps.tile([128, W], F32, name="ops")
                    for p in range(2):
                        nc.tensor.matmul(
                            o_ps[:],
                            lhsT=ht_sb[:, p * S + s * 128:p * S + (s + 1) * 128],
                            rhs=wo_sb[:, p * D + n * W:p * D + (n + 1) * W],
                            start=(p == 0), stop=(p == 1),
                        )
                    nc.vector.tensor_copy(o_sb[:, n * W:(n + 1) * W], o_ps[:])
                nc.sync.dma_start(part[s * 128:(s + 1) * 128, :], o_sb[:])
            return t

        hp = None
        for bI in range(2):
            for h in range(HG):
                pt, hi = h // 2, h % 2
                qbase = pt * S + bI * HALF
                if hi == 0:
                    hp = hpp.tile([128, HALF], BF16, name="hp")
                hu = [hups.tile([128, 260], F32, name=f"hu{g}") for g in range(2)]
                ets = [None] * NSK

                def emit_pv(sk, hu=hu, ets=ets):
                    for qt in range(8):
                        nc.tensor.matmul(
                            hu[qt // 4][:, (qt % 4) * 65:(qt % 4) * 65 + 65],
                            lhsT=ets[sk][:, qt * 128:(qt + 1) * 128],
                            rhs=v1_sb[:, sk * 65:(sk + 1) * 65],
                            # start zeroes the whole 2KB PSUM bank (all 4 qt
                            # regions), so only the bank's first/last matmul
                            # opens/closes the accumulation group
                            start=(sk == 0 and qt % 4 == 0),
                            stop=(sk == NSK - 1 and qt % 4 == 3),
                        )

                for sk in range(NSK):
                    if sk >= 2:
                        emit_pv(sk - 2)
                    sc = scps.tile([128, HALF], F32, name="sc")
                    for n in range(2):
                        nc.tensor.matmul(
                            sc[:, n * W:(n + 1) * W],
                            lhsT=kt2_sb[hi * 64:(hi + 1) * 64, sk * 128:(sk + 1) * 128],
                            rhs=qt_sb[hi * 64:(hi + 1) * 64, qbase + n * W:qbase + (n + 1) * W],
                            start=True, stop=True,
                        )
                    et = expp.tile([128, HALF], BF16, name="et")
                    nc.scalar.activation(et[:], sc[:], EXP, scale=0.125)
                    ets[sk] = et
                    if tasks:
                        tasks.popleft()()
                emit_pv(NSK - 2)
                emit_pv(NSK - 1)

                # normalize on eviction: hp[q, hi*64+d] = h_un[q, d] / sumexp[q]
                rec = smalls.tile([128, 8], F32, name="rec")
                for g in range(2):
                    for q4 in range(4):
                        nc.vector.reciprocal(
                            rec[:, g * 4 + q4:g * 4 + q4 + 1],
                            hu[g][:, q4 * 65 + 64:q4 * 65 + 65],
                        )
                for qt in range(8):
                    nc.vector.tensor_scalar_mul(
                        hp[:, qt * 128 + hi * 64:qt * 128 + hi * 64 + 64],
                        hu[qt // 4][:, (qt % 4) * 65:(qt % 4) * 65 + 64],
                        rec[:, qt:qt + 1],
                    )
                if hi == 1:
                    dst = ht_sb[:, pt * S + bI * HALF:pt * S + (bI + 1) * HALF]
                    dst = dst.rearrange("p (a b) -> p a b", b=128)
                    nc.sync.dma_start_transpose(dst, hp[:])
                    if pt == 1:
                        for sc_i in range(HALF // 128):
                            tasks.append(mk_outproj(bI, sc_i))
        while tasks:
            tasks.popleft()()
        outps_stack.close()
        if _DEBUG:
            nc.sync.dma_start(dbg["d_qt"][:, :], qt_sb[:])
            nc.sync.dma_start(dbg["d_kt2"][:, :], kt2_sb[:])
            nc.sync.dma_start(dbg["d_v1"][:, :], v1_sb[:])
            nc.sync.dma_start(dbg["d_ht"][:, :], ht_sb[:])

    nc.finalize()
    return nc


def _get_nc():
    if "nc" not in _CACHE:
        _CACHE["nc"] = _build_nc()
    return _CACHE["nc"]


def _prep_core_inputs(inputs, wq, bq, wk, wv, wo):
    """Host-side shard prep: per-core transposed/rearranged bf16 operands."""
    from ml_dtypes import bfloat16

    xT = [np.ascontiguousarray(np.asarray(inputs[b], np.float32).T).astype(bfloat16)
          for b in range(B)]
    wq3 = np.asarray(wq, np.float32).reshape(Dh, NUM_HEADS, D)
    bq2 = np.asarray(bq, np.float32).reshape(Dh, NUM_HEADS)
    wkvT = np.ascontiguousarray(
        np.concatenate([np.asarray(wk, np.float32).T, np.asarray(wv, np.float32).T],
                       axis=1)
    ).astype(bfloat16)  # [1024, 128], K in cols 0:64
    wo_ = np.asarray(wo, np.float32)

    in_maps = []
    for c in range(N_CORES):
        b, g = divmod(c, G)
        heads = [g * HG + hl for hl in range(HG)]
        wqT_g = np.ascontiguousarray(
            np.concatenate([wq3[:, h, :].T for h in heads], axis=1)
        ).astype(bfloat16)
        bq_g = np.ascontiguousarray(
            np.concatenate([bq2[:, h] for h in heads]).reshape(QD, 1).astype(np.float32)
        )
        woT_g = np.ascontiguousarray(
            wo_[:, g * QD:(g + 1) * QD].T
        ).astype(bfloat16)  # [256, 1024]
        in_maps.append({
            "xT": xT[b],
            "wqT": wqT_g,
            "wkvT": wkvT,
            "woT": woT_g,
            "bq": bq_g,
        })
    return in_maps


def kernel(inputs, wq, bq, wk, bk, wv, bv, wo, bo):
    from concourse.bass_utils import run_bass_kernel_spmd

    nc = _get_nc()
    in_maps = _prep_core_inputs(inputs, wq, bq, wk, wv, wo)
    res = run_bass_kernel_spmd(nc, in_maps, list(range(N_CORES))).results

    wo_ = np.asarray(wo, np.float32)
    bias = (
        np.asarray(bo, np.float32)
        + wo_ @ np.tile(np.asarray(bv, np.float32), NUM_HEADS)
    )
    out = np.empty((B, S, D), np.float32)
    for b in range(B):
        acc = res[b * G]["part"].astype(np.float32).copy()
        for g in range(1, G):
            acc += res[b * G + g]["part"]
        out[b] = acc + bias
    return out


# revision 29
# speedup vs baseline: 1.2078x; 1.0783x over previous
"""MQA self-attention kernel for Trainium2, 8 NeuronCores.

Reference computation (fp32):
    q = x @ wq.T + bq        -> [B,S,1024] -> heads via (hidden num_heads) split
    k = x @ wk.T + bk        -> [B,S,64]  (single shared KV head)
    v = x @ wv.T + bv
    scores = q @ k.T / 8 ; attn = softmax(scores) ; h = attn @ v
    out = merge_heads(h) @ wo.T + bo

Sharding (8 cores, no collectives): core c handles batch b=c//4 and head
group g=c%4 (4 of the 16 q-heads).  The shared K/V head is replicated.
Each core returns the partial output h_g @ wo_g.T [S, D]; the host sums
the 4 head-group partials per batch and adds the bias terms.

Math notes:
 - bk provably cancels in softmax; bv is folded into the host-side output
   bias (softmax rows sum to 1); softmax runs without max subtraction
   (scores ~ N(0,1), exp stays within bf16/f32 range).

Device pipeline (all operands bf16, PSUM f32; the Activation engine's
exp throughput ~133us/core is the hard floor, so every other engine is
kept strictly below it):
 - xT/weights land as bf16 (halves the input DMA), projections produce
   QT [2 heads stacked per 128 partitions], KT (two partition-offset
   copies so odd/even heads both get offset-matched operands) and VT.
 - V' = [V|1] built by PE-transpose (half 0) / DMA-transpose (half 1).
 - scores_T[k, q] per head per 1024-query block; exp on ScalarE in
   [128,1024] blocks writing bf16.
 - PV runs dense: h_un[q, 65] += exp_T[:, qtile].T @ V' accumulated over
   key tiles in PSUM (2x fewer PE rows than the hT-layout alternative).
 - normalize on eviction via per-partition reciprocal multiply; pairs of
   heads share an SBUF buffer that one DMA-transpose per pair flips into
   hT layout for the output projection.
 - half-1 projections and out-projection chunks are emitted as deferred
   PE tasks, one per exp slot, so the Tensor engine queue always has
   work but never starves the exp pipeline.
"""

from collections import deque

import numpy as np

NUM_HEADS = 16
Dh = 64
B, S, D = 2, 2048, 1024
G = 4            # head groups (cores per batch)
HG = 4           # heads per group
QD = HG * Dh     # 256 local q dims
NK = D // 128    # 8 contraction tiles for projections
NSK = S // 128   # 16 key tiles
W = 512          # matmul moving width
HALF = 1024      # query block / projection column half
N_CORES = 8

_CACHE = {}
_DEBUG = False


def _build_nc():
    from contextlib import ExitStack

    import concourse.bass as bass
    import concourse.mybir as mybir
    import concourse.tile as tile
    from concourse import bacc
    from concourse.masks import make_identity

    F32 = mybir.dt.float32
    BF16 = mybir.dt.bfloat16
    EXP = mybir.ActivationFunctionType.Exp

    nc = bacc.Bacc("TRN2", target_bir_lowering=False, debug=False)

    xT = nc.declare_dram_parameter("xT", [D, S], BF16, isOutput=False)
    wqT = nc.declare_dram_parameter("wqT", [D, QD], BF16, isOutput=False)
    wkvT = nc.declare_dram_parameter("wkvT", [D, 128], BF16, isOutput=False)
    woT = nc.declare_dram_parameter("woT", [QD, D], BF16, isOutput=False)
    bqp = nc.declare_dram_parameter("bq", [QD, 1], F32, isOutput=False)
    part = nc.declare_dram_parameter("part", [S, D], BF16, isOutput=True)
    if _DEBUG:
        dbg = {
            "d_qt": nc.declare_dram_parameter("d_qt", [128, 2 * S], BF16, isOutput=True),
            "d_kt2": nc.declare_dram_parameter("d_kt2", [128, S], BF16, isOutput=True),
            "d_v1": nc.declare_dram_parameter("d_v1", [128, NSK * 65], BF16, isOutput=True),
            "d_ht": nc.declare_dram_parameter("d_ht", [128, 2 * S], BF16, isOutput=True),
        }

    with tile.TileContext(nc) as tc, ExitStack() as ctx:
        const = ctx.enter_context(tc.tile_pool(name="const", bufs=1))
        persist = ctx.enter_context(tc.tile_pool(name="persist", bufs=1))

        wq_sb = const.tile([128, NK * QD], BF16)    # ktile kt at cols [kt*QD:+QD]
        wkv_sb = const.tile([128, NK * 128], BF16)  # cols 0:64 = wkT, 64:128 = wvT
        wo_sb = const.tile([128, 2 * D], BF16)      # q-ktile p at cols [p*D:+D]
        bq_sb = const.tile([128, 2], F32)
        ident = const.tile([128, 128], BF16)

        qt_sb = persist.tile([128, 2 * S], BF16)    # pt p cols [p*S:+S]; rows 0:64 head 2p, 64:128 head 2p+1
        kt2_sb = persist.tile([128, S], BF16)       # KT duplicated rows 0:64 and 64:128
        vt_sb = persist.tile([128, S], BF16)        # VT in rows 64:128
        v1_sb = persist.tile([128, NSK * 65], BF16)  # V' tile sk at cols [sk*65:+65]
        ht_sb = persist.tile([128, 2 * S], BF16)    # hT, q-ktile p at cols [p*S:+S]

        make_identity(nc, ident[:])
        nc.vector.memset(v1_sb[:], 1.0)  # pre-fill the softmax-denominator columns

        # ---- DMAs + half-0 projections --------------------------------
        # x lands in 4 tiles of 2 ktiles (half 0, fine overlap with the
        # projection chains) + 2 tiles of 4 ktiles (half 1).  Issue queues
        # are spread over SP/ACT/DVE so the ~1.2us per-DMA issue cost
        # doesn't serialize the critical path.
        xp = ctx.enter_context(tc.tile_pool(name="xp", bufs=1))
        x0 = [xp.tile([128, 2 * HALF], BF16, name=f"x0_{j}") for j in range(4)]
        x1 = [xp.tile([128, 4 * HALF], BF16, name=f"x1_{j}") for j in range(2)]

        nc.sync.dma_start(
            wkv_sb[:].rearrange("p (k c) -> p k c", c=128),
            wkvT[:, :].rearrange("(k p) c -> p k c", p=128),
        )
        nc.sync.dma_start(
            wq_sb[:].rearrange("p (k c) -> p k c", c=QD),
            wqT[:, :].rearrange("(k p) c -> p k c", p=128),
        )
        for j in range(4):
            nc.scalar.dma_start(
                x0[j][:].rearrange("p (k c) -> p k c", c=HALF),
                xT[j * 256:(j + 1) * 256, 0:HALF].rearrange("(k p) c -> p k c", p=128),
            )
        for j in range(2):
            nc.scalar.dma_start(
                x1[j][:].rearrange("p (k c) -> p k c", c=HALF),
                xT[j * 512:(j + 1) * 512, HALF:S].rearrange("(k p) c -> p k c", p=128),
            )
        for p in range(2):
            nc.sync.dma_start(bq_sb[:, p:p + 1], bqp[p * 128:(p + 1) * 128, :])
        nc.sync.dma_start(
            wo_sb[:].rearrange("p (a c) -> p a c", c=D),
            woT[:, :].rearrange("(a p) c -> p a c", p=128),
        )

        def xcol(kt, hf, n):
            if hf == 0:
                return x0[kt // 2][:, (kt % 2) * HALF + n * W:(kt % 2) * HALF + (n + 1) * W]
            return x1[kt // 4][:, (kt % 4) * HALF + n * W:(kt % 4) * HALF + (n + 1) * W]

        def proj_kt_step(ps, wsb_col, kt, hf, width):
            for n in range(HALF // W):
                nc.tensor.matmul(
                    ps[:, n * W:(n + 1) * W],
                    lhsT=wsb_col(kt),
                    rhs=xcol(kt, hf, n),
                    start=(kt == 0), stop=(kt == NK - 1),
                )

        wkv_col = lambda kt: wkv_sb[:, kt * 128:(kt + 1) * 128]
        wq0_col = lambda kt: wq_sb[:, kt * QD:kt * QD + 128]
        wq1_col = lambda kt: wq_sb[:, kt * QD + 128:kt * QD + 256]

        with (
            tc.tile_pool(name="p0", bufs=1, space="PSUM") as p0,
            tc.tile_pool(name="trps", bufs=2, space="PSUM") as trps,
        ):
            vk_ps = p0.tile([128, HALF], F32, name="vk")
            q0_ps = p0.tile([128, HALF], F32, name="q0")
            q1_ps = p0.tile([128, HALF], F32, name="q1")
            # vk + q0 gate the first exp; q1 (head 1) can trail.
            for kt in range(NK):
                proj_kt_step(vk_ps, wkv_col, kt, 0, HALF)
                proj_kt_step(q0_ps, wq0_col, kt, 0, HALF)
            nc.scalar.copy(kt2_sb[0:64, 0:HALF], vk_ps[0:64, :])
            nc.vector.tensor_scalar_add(qt_sb[:, 0:HALF], q0_ps[:], bq_sb[:, 0:1])
            for kt in range(NK):
                proj_kt_step(q1_ps, wq1_col, kt, 0, HALF)
            nc.vector.tensor_copy(vt_sb[64:128, 0:HALF], vk_ps[64:128, :])
            nc.gpsimd.tensor_copy(kt2_sb[64:128, 0:HALF], kt2_sb[0:64, 0:HALF])
            nc.vector.tensor_scalar_add(qt_sb[:, S:S + HALF], q1_ps[:], bq_sb[:, 1:2])
            # V' half 0 by PE transpose (DMA engines are busy with x half 1)
            for sk in range(NSK // 2):
                tr = trps.tile([128, Dh], BF16, name="tr")
                nc.tensor.transpose(
                    tr[:], vt_sb[64:128, sk * 128:(sk + 1) * 128],
                    ident[64:128, 64:128],
                )
                nc.vector.tensor_copy(v1_sb[:, sk * 65:sk * 65 + 64], tr[:])

        # ---- Phase 2: attention, with deferred fill-in PE tasks -------
        scps = ctx.enter_context(tc.tile_pool(name="scps", bufs=2, space="PSUM"))
        hups = ctx.enter_context(tc.tile_pool(name="hups", bufs=1, space="PSUM"))
        expp = ctx.enter_context(tc.tile_pool(name="expp", bufs=7))
        hpp = ctx.enter_context(tc.tile_pool(name="hpp", bufs=2))
        smalls = ctx.enter_context(tc.tile_pool(name="smalls", bufs=4))
        osbp = ctx.enter_context(tc.tile_pool(name="osbp", bufs=2))

        state = {}

        def mk_vk1(kts):
            def t():
                if "vk1" not in state:
                    pvk = pvk_stack.enter_context(
                        tc.tile_pool(name="pvk", bufs=1, space="PSUM"))
                    state["vk1"] = pvk.tile([128, HALF], F32, name="vkps")
                for kt in kts:
                    proj_kt_step(state["vk1"], wkv_col, kt, 1, HALF)
            return t

        def t_kv1_evict():
            vk1 = state.pop("vk1")
            nc.vector.tensor_copy(kt2_sb[0:64, HALF:S], vk1[0:64, :])
            nc.vector.tensor_copy(vt_sb[64:128, HALF:S], vk1[64:128, :])
            nc.gpsimd.tensor_copy(kt2_sb[64:128, HALF:S], kt2_sb[0:64, HALF:S])
            pvk_stack.close()

        def t_v1_h1():
            # V' half 1 by PE transpose, in a short-lived 1-bank PSUM era
            # (dma_start_transpose mis-executes the 65-strided pattern on HW)
            with tc.tile_pool(name="trps1", bufs=2, space="PSUM") as trps1:
                for sk in range(NSK // 2, NSK):
                    tr = trps1.tile([128, Dh], BF16, name="tr1")
                    nc.tensor.transpose(
                        tr[:], vt_sb[64:128, sk * 128:(sk + 1) * 128],
                        ident[64:128, 64:128],
                    )
                    nc.vector.tensor_copy(v1_sb[:, sk * 65:sk * 65 + 64], tr[:])

        def mk_q1(which, col_fn, kts):
            def t():
                if which in state:
                    ps = state[which]
                else:
                    if "pq" not in state:
                        state["pq"] = pq_stack.enter_context(
                            tc.tile_pool(name="pq", bufs=1, space="PSUM"))
                    ps = state[which] = state["pq"].tile([128, HALF], F32, name="qps")
                for kt in kts:
                    proj_kt_step(ps, col_fn, kt, 1, HALF)
            return t

        def mk_q1_evict(which, pt):
            def t():
                ps = state.pop(which)
                nc.vector.tensor_scalar_add(
                    qt_sb[:, pt * S + HALF:pt * S + S], ps[:], bq_sb[:, pt:pt + 1]
                )
            return t

        def t_close_p1b():
            pq_stack.close()

        tasks = deque([
            mk_vk1(range(0, 2)), mk_vk1(range(2, 4)),
            mk_vk1(range(4, 6)), mk_vk1(range(6, 8)),
            t_kv1_evict, t_v1_h1,
            mk_q1("q0h1", wq0_col, range(0, 2)), mk_q1("q0h1", wq0_col, range(2, 4)),
            mk_q1("q0h1", wq0_col, range(4, 6)), mk_q1("q0h1", wq0_col, range(6, 8)),
            mk_q1_evict("q0h1", 0),
            mk_q1("q1h1", wq1_col, range(0, 2)), mk_q1("q1h1", wq1_col, range(2, 4)),
            mk_q1("q1h1", wq1_col, range(4, 6)), mk_q1("q1h1", wq1_col, range(6, 8)),
            mk_q1_evict("q1h1", 1),
            t_close_p1b,
        ])

        outps_stack = ExitStack()
        outps = None
        pvk_stack = ExitStack()
        pq_stack = ExitStack()

        def mk_outproj(bI, sc_i):
            def t():
                nonlocal outps
                if outps is None:
                    outps = outps_stack.enter_context(
                        tc.tile_pool(name="outps", bufs=2, space="PSUM"))
                s = bI * (HALF // 128) + sc_i
                half = sc_i % 2
                if half == 0:
                    state["osb"] = osbp.tile([128, 2 * D], BF16, name="osb")
                o_sb = state["osb"]
                for n in range(2):
                    o_ps = outps.tile([128, W], F32, name="ops")
                    for p in range(2):
                        nc.tensor.matmul(
                            o_ps[:],
                            lhsT=ht_sb[:, p * S + s * 128:p * S + (s + 1) * 128],
                            rhs=wo_sb[:, p * D + n * W:p * D + (n + 1) * W],
                            start=(p == 0), stop=(p == 1),
                        )
                    nc.vector.tensor_copy(
                        o_sb[:, half * D + n * W:half * D + (n + 1) * W], o_ps[:])
                if half == 1:
                    nc.sync.dma_start(
                        part[(s - 1) * 128:(s + 1) * 128, :].rearrange(
                            "(c p) d -> p c d", p=128),
                        o_sb[:].rearrange("p (c d) -> p c d", d=D),
                    )
            return t

        hp = None
        for bI in range(2):
            for h in range(HG):
                pt, hi = h // 2, h % 2
                qbase = pt * S + bI * HALF
                if hi == 0:
                    hp = hpp.tile([128, HALF], BF16, name="hp")
                hu = [hups.tile([128, 260], F32, name=f"hu{g}") for g in range(2)]
                ets = [None] * NSK

                def emit_pv(sk, hu=hu, ets=ets):
                    for qt in range(8):
                        nc.tensor.matmul(
                            hu[qt // 4][:, (qt % 4) * 65:(qt % 4) * 65 + 65],
                            lhsT=ets[sk][:, qt * 128:(qt + 1) * 128],
                            rhs=v1_sb[:, sk * 65:(sk + 1) * 65],
                            # start zeroes the whole 2KB PSUM bank (all 4 qt
                            # regions), so only the bank's first/last matmul
                            # opens/closes the accumulation group
                            start=(sk == 0 and qt % 4 == 0),
                            stop=(sk == NSK - 1 and qt % 4 == 3),
                        )

                for sk in range(NSK):
                    # PV trails exp by 4 slots: the first PV of a head waits
                    # on the previous head's norm-eviction freeing the hu
                    # banks, and a deeper pipeline keeps that wait off the
                    # in-order PE queue's critical path.
                    if sk >= 4:
                        emit_pv(sk - 4)
                    sc = scps.tile([128, HALF], F32, name="sc")
                    for n in range(2):
                        nc.tensor.matmul(
                            sc[:, n * W:(n + 1) * W],
                            lhsT=kt2_sb[hi * 64:(hi + 1) * 64, sk * 128:(sk + 1) * 128],
                            rhs=qt_sb[hi * 64:(hi + 1) * 64, qbase + n * W:qbase + (n + 1) * W],
                            start=True, stop=True,
                        )
                    et = expp.tile([128, HALF], BF16, name="et")
                    nc.scalar.activation(et[:], sc[:], EXP, scale=0.125)
                    ets[sk] = et
                    if tasks:
                        tasks.popleft()()
                for sk in range(NSK - 4, NSK):
                    emit_pv(sk)

                # normalize on eviction: hp[q, hi*64+d] = h_un[q, d] / sumexp[q]
                rec = smalls.tile([128, 8], F32, name="rec")
                for g in range(2):
                    for q4 in range(4):
                        nc.vector.reciprocal(
                            rec[:, g * 4 + q4:g * 4 + q4 + 1],
                            hu[g][:, q4 * 65 + 64:q4 * 65 + 65],
                        )
                for qt in range(8):
                    nc.vector.tensor_scalar_mul(
                        hp[:, qt * 128 + hi * 64:qt * 128 + hi * 64 + 64],
                        hu[qt // 4][:, (qt % 4) * 65:(qt % 4) * 65 + 64],
                        rec[:, qt:qt + 1],
                    )
                if hi == 1:
                    dst = ht_sb[:, pt * S + bI * HALF:pt * S + (bI + 1) * HALF]
                    dst = dst.rearrange("p (a b) -> p a b", b=128)
                    nc.sync.dma_start_transpose(dst, hp[:])
                    if pt == 1:
                        for sc_i in range(HALF // 128):
                            tasks.append(mk_outproj(bI, sc_i))
        while tasks:
            tasks.popleft()()
        outps_stack.close()
        if _DEBUG:
            nc.sync.dma_start(dbg["d_qt"][:, :], qt_sb[:])
            nc.sync.dma_start(dbg["d_kt2"][:, :], kt2_sb[:])
            nc.sync.dma_start(dbg["d_v1"][:, :], v1_sb[:])
            nc.sync.dma_start(dbg["d_ht"][:, :], ht_sb[:])

    nc.finalize()
    return nc


def _get_nc():
    if "nc" not in _CACHE:
        _CACHE["nc"] = _build_nc()
    return _CACHE["nc"]


def _prep_core_inputs(inputs, wq, bq, wk, wv, wo):
    """Host-side shard prep: per-core transposed/rearranged bf16 operands."""
    from ml_dtypes import bfloat16

    xT = [np.ascontiguousarray(np.asarray(inputs[b], np.float32).T).astype(bfloat16)
          for b in range(B)]
    wq3 = np.asarray(wq, np.float32).reshape(Dh, NUM_HEADS, D)
    bq2 = np.asarray(bq, np.float32).reshape(Dh, NUM_HEADS)
    wkvT = np.ascontiguousarray(
        np.concatenate([np.asarray(wk, np.float32).T, np.asarray(wv, np.float32).T],
                       axis=1)
    ).astype(bfloat16)  # [1024, 128], K in cols 0:64
    wo_ = np.asarray(wo, np.float32)

    in_maps = []
    for c in range(N_CORES):
        b, g = divmod(c, G)
        heads = [g * HG + hl for hl in range(HG)]
        wqT_g = np.ascontiguousarray(
            np.concatenate([wq3[:, h, :].T for h in heads], axis=1)
        ).astype(bfloat16)
        bq_g = np.ascontiguousarray(
            np.concatenate([bq2[:, h] for h in heads]).reshape(QD, 1).astype(np.float32)
        )
        woT_g = np.ascontiguousarray(
            wo_[:, g * QD:(g + 1) * QD].T
        ).astype(bfloat16)  # [256, 1024]
        in_maps.append({
            "xT": xT[b],
            "wqT": wqT_g,
            "wkvT": wkvT,
            "woT": woT_g,
            "bq": bq_g,
        })
    return in_maps


def kernel(inputs, wq, bq, wk, bk, wv, bv, wo, bo):
    from concourse.bass_utils import run_bass_kernel_spmd

    nc = _get_nc()
    in_maps = _prep_core_inputs(inputs, wq, bq, wk, wv, wo)
    res = run_bass_kernel_spmd(nc, in_maps, list(range(N_CORES))).results

    wo_ = np.asarray(wo, np.float32)
    bias = (
        np.asarray(bo, np.float32)
        + wo_ @ np.tile(np.asarray(bv, np.float32), NUM_HEADS)
    )
    out = np.empty((B, S, D), np.float32)
    for b in range(B):
        acc = res[b * G]["part"].astype(np.float32).copy()
        for g in range(1, G):
            acc += res[b * G + g]["part"]
        out[b] = acc + bias
    return out


# revision 32
# speedup vs baseline: 1.2248x; 1.0141x over previous
"""MQA self-attention kernel for Trainium2, 8 NeuronCores.

Reference computation (fp32):
    q = x @ wq.T + bq        -> [B,S,1024] -> heads via (hidden num_heads) split
    k = x @ wk.T + bk        -> [B,S,64]  (single shared KV head)
    v = x @ wv.T + bv
    scores = q @ k.T / 8 ; attn = softmax(scores) ; h = attn @ v
    out = merge_heads(h) @ wo.T + bo

Sharding (8 cores, no collectives): core c handles batch b=c//4 and head
group g=c%4 (4 of the 16 q-heads).  The shared K/V head is replicated.
Each core returns the partial output h_g @ wo_g.T [S, D]; the host sums
the 4 head-group partials per batch and adds the bias terms.

Math notes:
 - bk provably cancels in softmax; bv is folded into the host-side output
   bias (softmax rows sum to 1); softmax runs without max subtraction
   (scores ~ N(0,1), exp stays within bf16/f32 range).

Device pipeline (all operands bf16, PSUM f32; the Activation engine's
exp throughput ~133us/core is the hard floor, so every other engine is
kept strictly below it):
 - xT/weights land as bf16 (halves the input DMA), projections produce
   QT [2 heads stacked per 128 partitions], KT (two partition-offset
   copies so odd/even heads both get offset-matched operands) and VT.
 - V' = [V|1] built by PE-transpose (half 0) / DMA-transpose (half 1).
 - scores_T[k, q] per head per 1024-query block; exp on ScalarE in
   [128,1024] blocks writing bf16.
 - PV runs dense: h_un[q, 65] += exp_T[:, qtile].T @ V' accumulated over
   key tiles in PSUM (2x fewer PE rows than the hT-layout alternative).
 - normalize on eviction via per-partition reciprocal multiply; pairs of
   heads share an SBUF buffer that one DMA-transpose per pair flips into
   hT layout for the output projection.
 - half-1 projections and out-projection chunks are emitted as deferred
   PE tasks, one per exp slot, so the Tensor engine queue always has
   work but never starves the exp pipeline.
"""

from collections import deque

import numpy as np

NUM_HEADS = 16
Dh = 64
B, S, D = 2, 2048, 1024
G = 4            # head groups (cores per batch)
HG = 4           # heads per group
QD = HG * Dh     # 256 local q dims
NK = D // 128    # 8 contraction tiles for projections
NSK = S // 128   # 16 key tiles
W = 512          # matmul moving width
HALF = 1024      # query block / projection column half
N_CORES = 8

_CACHE = {}
_DEBUG = False


def _build_nc():
    from contextlib import ExitStack

    import concourse.bass as bass
    import concourse.mybir as mybir
    import concourse.tile as tile
    from concourse import bacc
    from concourse.masks import make_identity

    F32 = mybir.dt.float32
    BF16 = mybir.dt.bfloat16
    EXP = mybir.ActivationFunctionType.Exp

    nc = bacc.Bacc("TRN2", target_bir_lowering=False, debug=False)

    xT = nc.declare_dram_parameter("xT", [D, S], BF16, isOutput=False)
    wqT = nc.declare_dram_parameter("wqT", [D, QD], BF16, isOutput=False)
    wkvT = nc.declare_dram_parameter("wkvT", [D, 128], BF16, isOutput=False)
    woT = nc.declare_dram_parameter("woT", [QD, D], BF16, isOutput=False)
    bqp = nc.declare_dram_parameter("bq", [QD, 1], F32, isOutput=False)
    part = nc.declare_dram_parameter("part", [S, D], BF16, isOutput=True)
    if _DEBUG:
        dbg = {
            "d_qt": nc.declare_dram_parameter("d_qt", [128, 2 * S], BF16, isOutput=True),
            "d_kt2": nc.declare_dram_parameter("d_kt2", [128, S], BF16, isOutput=True),
            "d_v1": nc.declare_dram_parameter("d_v1", [128, NSK * 65], BF16, isOutput=True),
            "d_ht": nc.declare_dram_parameter("d_ht", [128, 2 * S], BF16, isOutput=True),
        }

    with tile.TileContext(nc) as tc, ExitStack() as ctx:
        const = ctx.enter_context(tc.tile_pool(name="const", bufs=1))
        persist = ctx.enter_context(tc.tile_pool(name="persist", bufs=1))

        wq_sb = const.tile([128, NK * QD], BF16)    # ktile kt at cols [kt*QD:+QD]
        wkv_sb = const.tile([128, NK * 128], BF16)  # cols 0:64 = wkT, 64:128 = wvT
        wo_sb = const.tile([128, 2 * D], BF16)      # q-ktile p at cols [p*D:+D]
        bq_sb = const.tile([128, 2], F32)
        ident = const.tile([128, 128], BF16)

        qt_sb = persist.tile([128, 2 * S], BF16)    # pt p cols [p*S:+S]; rows 0:64 head 2p, 64:128 head 2p+1
        kt2_sb = persist.tile([128, S], BF16)       # KT duplicated rows 0:64 and 64:128
        vt_sb = persist.tile([128, S], BF16)        # VT in rows 64:128
        v1_sb = persist.tile([128, NSK * 65], BF16)  # V' tile sk at cols [sk*65:+65]
        ht_sb = persist.tile([128, 2 * S], BF16)    # hT, q-ktile p at cols [p*S:+S]

        make_identity(nc, ident[:])
        nc.vector.memset(v1_sb[:], 1.0)  # pre-fill the softmax-denominator columns

        # ---- DMAs + half-0 projections --------------------------------
        # x lands in 4 tiles of 2 ktiles (half 0, fine overlap with the
        # projection chains) + 2 tiles of 4 ktiles (half 1).  Issue queues
        # are spread over SP/ACT/DVE so the ~1.2us per-DMA issue cost
        # doesn't serialize the critical path.
        xp = ctx.enter_context(tc.tile_pool(name="xp", bufs=1))
        x0 = [xp.tile([128, 2 * HALF], BF16, name=f"x0_{j}") for j in range(4)]
        x1 = [xp.tile([128, 2 * HALF], BF16, name=f"x1_{j}") for j in range(4)]

        def dma_x(eng, tiles, j, hf):
            eng.dma_start(
                tiles[j][:].rearrange("p (k c) -> p k c", c=HALF),
                xT[j * 256:(j + 1) * 256, hf * HALF:(hf + 1) * HALF].rearrange(
                    "(k p) c -> p k c", p=128),
            )

        nc.sync.dma_start(
            wkv_sb[:].rearrange("p (k c) -> p k c", c=128),
            wkvT[:, :].rearrange("(k p) c -> p k c", p=128),
        )
        nc.sync.dma_start(
            wq_sb[:].rearrange("p (k c) -> p k c", c=QD),
            wqT[:, :].rearrange("(k p) c -> p k c", p=128),
        )
        # split issues over the SP and ACT queues: ~1.2-2us of serialized
        # issue cost per DMA would otherwise gate the first exp
        dma_x(nc.sync, x0, 0, 0)
        dma_x(nc.scalar, x0, 1, 0)
        dma_x(nc.sync, x0, 2, 0)
        dma_x(nc.scalar, x0, 3, 0)
        dma_x(nc.sync, x1, 0, 1)
        dma_x(nc.scalar, x1, 1, 1)
        dma_x(nc.sync, x1, 2, 1)
        dma_x(nc.scalar, x1, 3, 1)
        for p in range(2):
            nc.sync.dma_start(bq_sb[:, p:p + 1], bqp[p * 128:(p + 1) * 128, :])
        nc.sync.dma_start(
            wo_sb[:].rearrange("p (a c) -> p a c", c=D),
            woT[:, :].rearrange("(a p) c -> p a c", p=128),
        )

        def xcol(kt, hf, n):
            tiles = x0 if hf == 0 else x1
            return tiles[kt // 2][:, (kt % 2) * HALF + n * W:(kt % 2) * HALF + (n + 1) * W]

        def proj_kt_step(ps, wsb_col, kt, hf, width):
            for n in range(HALF // W):
                nc.tensor.matmul(
                    ps[:, n * W:(n + 1) * W],
                    lhsT=wsb_col(kt),
                    rhs=xcol(kt, hf, n),
                    start=(kt == 0), stop=(kt == NK - 1),
                )

        wkv_col = lambda kt: wkv_sb[:, kt * 128:(kt + 1) * 128]
        wq0_col = lambda kt: wq_sb[:, kt * QD:kt * QD + 128]
        wq1_col = lambda kt: wq_sb[:, kt * QD + 128:kt * QD + 256]

        with (
            tc.tile_pool(name="p0", bufs=1, space="PSUM") as p0,
            tc.tile_pool(name="trps", bufs=2, space="PSUM") as trps,
        ):
            vk_ps = p0.tile([128, HALF], F32, name="vk")
            q0_ps = p0.tile([128, HALF], F32, name="q0")
            q1_ps = p0.tile([128, HALF], F32, name="q1")
            # vk + q0 gate the first exp; q1 (head 1) can trail.
            for kt in range(NK):
                proj_kt_step(vk_ps, wkv_col, kt, 0, HALF)
                proj_kt_step(q0_ps, wq0_col, kt, 0, HALF)
            nc.scalar.copy(kt2_sb[0:64, 0:HALF], vk_ps[0:64, :])
            nc.vector.tensor_scalar_add(qt_sb[:, 0:HALF], q0_ps[:], bq_sb[:, 0:1])
            for kt in range(NK):
                proj_kt_step(q1_ps, wq1_col, kt, 0, HALF)
            nc.vector.tensor_copy(vt_sb[64:128, 0:HALF], vk_ps[64:128, :])
            nc.gpsimd.tensor_copy(kt2_sb[64:128, 0:HALF], kt2_sb[0:64, 0:HALF])
            nc.vector.tensor_scalar_add(qt_sb[:, S:S + HALF], q1_ps[:], bq_sb[:, 1:2])
            # V' half 0 by PE transpose (DMA engines are busy with x half 1)
            for sk in range(NSK // 2):
                tr = trps.tile([128, Dh], BF16, name="tr")
                nc.tensor.transpose(
                    tr[:], vt_sb[64:128, sk * 128:(sk + 1) * 128],
                    ident[64:128, 64:128],
                )
                nc.vector.tensor_copy(v1_sb[:, sk * 65:sk * 65 + 64], tr[:])

        # ---- Phase 2: attention, with deferred fill-in PE tasks -------
        scps = ctx.enter_context(tc.tile_pool(name="scps", bufs=2, space="PSUM"))
        hups = ctx.enter_context(tc.tile_pool(name="hups", bufs=1, space="PSUM"))
        expp = ctx.enter_context(tc.tile_pool(name="expp", bufs=7))
        hpp = ctx.enter_context(tc.tile_pool(name="hpp", bufs=2))
        smalls = ctx.enter_context(tc.tile_pool(name="smalls", bufs=4))
        osbp = ctx.enter_context(tc.tile_pool(name="osbp", bufs=2))

        state = {}

        def mk_vk1(kts):
            def t():
                if "vk1" not in state:
                    pvk = pvk_stack.enter_context(
                        tc.tile_pool(name="pvk", bufs=1, space="PSUM"))
                    state["vk1"] = pvk.tile([128, HALF], F32, name="vkps")
                for kt in kts:
                    proj_kt_step(state["vk1"], wkv_col, kt, 1, HALF)
            return t

        def t_kv1_evict():
            vk1 = state.pop("vk1")
            nc.vector.tensor_copy(kt2_sb[0:64, HALF:S], vk1[0:64, :])
            nc.vector.tensor_copy(vt_sb[64:128, HALF:S], vk1[64:128, :])
            nc.gpsimd.tensor_copy(kt2_sb[64:128, HALF:S], kt2_sb[0:64, HALF:S])
            pvk_stack.close()

        def t_v1_h1():
            # V' half 1 by PE transpose, in a short-lived 1-bank PSUM era
            # (dma_start_transpose mis-executes the 65-strided pattern on HW)
            with tc.tile_pool(name="trps1", bufs=2, space="PSUM") as trps1:
                for sk in range(NSK // 2, NSK):
                    tr = trps1.tile([128, Dh], BF16, name="tr1")
                    nc.tensor.transpose(
                        tr[:], vt_sb[64:128, sk * 128:(sk + 1) * 128],
                        ident[64:128, 64:128],
                    )
                    nc.vector.tensor_copy(v1_sb[:, sk * 65:sk * 65 + 64], tr[:])

        def mk_q1(which, col_fn, kts):
            def t():
                if which in state:
                    ps = state[which]
                else:
                    if "pq" not in state:
                        state["pq"] = pq_stack.enter_context(
                            tc.tile_pool(name="pq", bufs=1, space="PSUM"))
                    ps = state[which] = state["pq"].tile([128, HALF], F32, name="qps")
                for kt in kts:
                    proj_kt_step(ps, col_fn, kt, 1, HALF)
            return t

        def mk_q1_evict(which, pt):
            def t():
                ps = state.pop(which)
                nc.vector.tensor_scalar_add(
                    qt_sb[:, pt * S + HALF:pt * S + S], ps[:], bq_sb[:, pt:pt + 1]
                )
            return t

        def t_close_p1b():
            pq_stack.close()

        tasks = deque([
            mk_vk1(range(0, 2)), mk_vk1(range(2, 4)),
            mk_vk1(range(4, 6)), mk_vk1(range(6, 8)),
            t_kv1_evict, t_v1_h1,
            mk_q1("q0h1", wq0_col, range(0, 2)), mk_q1("q0h1", wq0_col, range(2, 4)),
            mk_q1("q0h1", wq0_col, range(4, 6)), mk_q1("q0h1", wq0_col, range(6, 8)),
            mk_q1_evict("q0h1", 0),
            mk_q1("q1h1", wq1_col, range(0, 2)), mk_q1("q1h1", wq1_col, range(2, 4)),
            mk_q1("q1h1", wq1_col, range(4, 6)), mk_q1("q1h1", wq1_col, range(6, 8)),
            mk_q1_evict("q1h1", 1),
            t_close_p1b,
        ])

        outps_stack = ExitStack()
        outps = None
        pvk_stack = ExitStack()
        pq_stack = ExitStack()

        def mk_outproj(bI, sc_i, n, p):
            # one matmul per task (~213ns) so out-projection fill-in never
            # blows the per-exp-slot PE budget
            def t():
                nonlocal outps
                if outps is None:
                    outps = outps_stack.enter_context(
                        tc.tile_pool(name="outps", bufs=2, space="PSUM"))
                s = bI * (HALF // 128) + sc_i
                half = sc_i % 2
                if half == 0 and n == 0 and p == 0:
                    state["osb"] = osbp.tile([128, 2 * D], BF16, name="osb")
                o_sb = state["osb"]
                if p == 0:
                    state["ops"] = outps.tile([128, W], F32, name="ops")
                o_ps = state["ops"]
                nc.tensor.matmul(
                    o_ps[:],
                    lhsT=ht_sb[:, p * S + s * 128:p * S + (s + 1) * 128],
                    rhs=wo_sb[:, p * D + n * W:p * D + (n + 1) * W],
                    start=(p == 0), stop=(p == 1),
                )
                if p == 1:
                    # the last block's evictions alternate DVE/ACT (ACT is
                    # done with exp by then); mid-kernel ones stay on DVE
                    dst = o_sb[:, half * D + n * W:half * D + (n + 1) * W]
                    if bI == 1 and n == 1:
                        nc.scalar.copy(dst, o_ps[:])
                    else:
                        nc.vector.tensor_copy(dst, o_ps[:])
                if half == 1 and n == 1 and p == 1:
                    nc.sync.dma_start(
                        part[(s - 1) * 128:(s + 1) * 128, :].rearrange(
                            "(c p) d -> p c d", p=128),
                        o_sb[:].rearrange("p (c d) -> p c d", d=D),
                    )
            return t

        hp = None
        for bI in range(2):
            for h in range(HG):
                pt, hi = h // 2, h % 2
                qbase = pt * S + bI * HALF
                if hi == 0:
                    hp = hpp.tile([128, HALF], BF16, name="hp")
                hu = [hups.tile([128, 260], F32, name=f"hu{g}") for g in range(2)]
                ets = [None] * NSK

                def emit_pv(sk, hu=hu, ets=ets):
                    for qt in range(8):
                        nc.tensor.matmul(
                            hu[qt // 4][:, (qt % 4) * 65:(qt % 4) * 65 + 65],
                            lhsT=ets[sk][:, qt * 128:(qt + 1) * 128],
                            rhs=v1_sb[:, sk * 65:(sk + 1) * 65],
                            # start zeroes the whole 2KB PSUM bank (all 4 qt
                            # regions), so only the bank's first/last matmul
                            # opens/closes the accumulation group
                            start=(sk == 0 and qt % 4 == 0),
                            stop=(sk == NSK - 1 and qt % 4 == 3),
                        )

                for sk in range(NSK):
                    # PV trails exp by 4 slots: the first PV of a head waits
                    # on the previous head's norm-eviction freeing the hu
                    # banks, and a deeper pipeline keeps that wait off the
                    # in-order PE queue's critical path.
                    if sk >= 4:
                        emit_pv(sk - 4)
                    sc = scps.tile([128, HALF], F32, name="sc")
                    for n in range(2):
                        nc.tensor.matmul(
                            sc[:, n * W:(n + 1) * W],
                            lhsT=kt2_sb[hi * 64:(hi + 1) * 64, sk * 128:(sk + 1) * 128],
                            rhs=qt_sb[hi * 64:(hi + 1) * 64, qbase + n * W:qbase + (n + 1) * W],
                            start=True, stop=True,
                        )
                    et = expp.tile([128, HALF], BF16, name="et")
                    nc.scalar.activation(et[:], sc[:], EXP, scale=0.125)
                    ets[sk] = et
                    if tasks:
                        tasks.popleft()()
                for sk in range(NSK - 4, NSK):
                    emit_pv(sk)

                # normalize on eviction: hp[q, hi*64+d] = h_un[q, d] / sumexp[q]
                rec = smalls.tile([128, 8], F32, name="rec")
                for g in range(2):
                    for q4 in range(4):
                        nc.vector.reciprocal(
                            rec[:, g * 4 + q4:g * 4 + q4 + 1],
                            hu[g][:, q4 * 65 + 64:q4 * 65 + 65],
                        )
                for qt in range(8):
                    nc.vector.tensor_scalar_mul(
                        hp[:, qt * 128 + hi * 64:qt * 128 + hi * 64 + 64],
                        hu[qt // 4][:, (qt % 4) * 65:(qt % 4) * 65 + 64],
                        rec[:, qt:qt + 1],
                    )
                    if hi == 1 and qt in (3, 7):
                        # transpose each 512-col half as soon as its four
                        # norm-evictions land: halves the transpose latency
                        # hanging off the last head's tail
                        lo = (qt - 3) * 128
                        dst = ht_sb[:, pt * S + bI * HALF + lo:
                                    pt * S + bI * HALF + lo + 4 * 128]
                        dst = dst.rearrange("p (a b) -> p a b", b=128)
                        nc.sync.dma_start_transpose(dst, hp[:, lo:lo + 4 * 128])
                if hi == 1 and pt == 1:
                    for sc_i in range(HALF // 128):
                        for n in range(2):
                            for p in range(2):
                                tasks.append(mk_outproj(bI, sc_i, n, p))
        while tasks:
            tasks.popleft()()
        outps_stack.close()
        if _DEBUG:
            nc.sync.dma_start(dbg["d_qt"][:, :], qt_sb[:])
            nc.sync.dma_start(dbg["d_kt2"][:, :], kt2_sb[:])
            nc.sync.dma_start(dbg["d_v1"][:, :], v1_sb[:])
            nc.sync.dma_start(dbg["d_ht"][:, :], ht_sb[:])

    nc.finalize()
    return nc


def _get_nc():
    if "nc" not in _CACHE:
        _CACHE["nc"] = _build_nc()
    return _CACHE["nc"]


def _prep_core_inputs(inputs, wq, bq, wk, wv, wo):
    """Host-side shard prep: per-core transposed/rearranged bf16 operands."""
    from ml_dtypes import bfloat16

    xT = [np.ascontiguousarray(np.asarray(inputs[b], np.float32).T).astype(bfloat16)
          for b in range(B)]
    wq3 = np.asarray(wq, np.float32).reshape(Dh, NUM_HEADS, D)
    bq2 = np.asarray(bq, np.float32).reshape(Dh, NUM_HEADS)
    wkvT = np.ascontiguousarray(
        np.concatenate([np.asarray(wk, np.float32).T, np.asarray(wv, np.float32).T],
                       axis=1)
    ).astype(bfloat16)  # [1024, 128], K in cols 0:64
    wo_ = np.asarray(wo, np.float32)

    in_maps = []
    for c in range(N_CORES):
        b, g = divmod(c, G)
        heads = [g * HG + hl for hl in range(HG)]
        wqT_g = np.ascontiguousarray(
            np.concatenate([wq3[:, h, :].T for h in heads], axis=1)
        ).astype(bfloat16)
        bq_g = np.ascontiguousarray(
            np.concatenate([bq2[:, h] for h in heads]).reshape(QD, 1).astype(np.float32)
        )
        woT_g = np.ascontiguousarray(
            wo_[:, g * QD:(g + 1) * QD].T
        ).astype(bfloat16)  # [256, 1024]
        in_maps.append({
            "xT": xT[b],
            "wqT": wqT_g,
            "wkvT": wkvT,
            "woT": woT_g,
            "bq": bq_g,
        })
    return in_maps


def kernel(inputs, wq, bq, wk, bk, wv, bv, wo, bo):
    from concourse.bass_utils import run_bass_kernel_spmd

    nc = _get_nc()
    in_maps = _prep_core_inputs(inputs, wq, bq, wk, wv, wo)
    res = run_bass_kernel_spmd(nc, in_maps, list(range(N_CORES))).results

    wo_ = np.asarray(wo, np.float32)
    bias = (
        np.asarray(bo, np.float32)
        + wo_ @ np.tile(np.asarray(bv, np.float32), NUM_HEADS)
    )
    out = np.empty((B, S, D), np.float32)
    for b in range(B):
        acc = res[b * G]["part"].astype(np.float32).copy()
        for g in range(1, G):
            acc += res[b * G + g]["part"]
        out[b] = acc + bias
    return out


# revision 38
# speedup vs baseline: 1.2521x; 1.0223x over previous
"""MQA self-attention kernel for Trainium2, 8 NeuronCores.

Reference computation (fp32):
    q = x @ wq.T + bq        -> [B,S,1024] -> heads via (hidden num_heads) split
    k = x @ wk.T + bk        -> [B,S,64]  (single shared KV head)
    v = x @ wv.T + bv
    scores = q @ k.T / 8 ; attn = softmax(scores) ; h = attn @ v
    out = merge_heads(h) @ wo.T + bo

Sharding (8 cores, no collectives): core c handles batch b=c//4 and head
group g=c%4 (4 of the 16 q-heads).  The shared K/V head is replicated.
Each core returns the partial output h_g @ wo_g.T [S, D]; the host sums
the 4 head-group partials per batch and adds the bias terms.

Math notes:
 - bk provably cancels in softmax; bv is folded into the host-side output
   bias (softmax rows sum to 1); softmax runs without max subtraction
   (scores ~ N(0,1), exp stays within bf16/f32 range).

Device pipeline (all operands bf16, PSUM f32; the Activation engine's
exp throughput ~133us/core is the hard floor, so every other engine is
kept strictly below it):
 - xT/weights land as bf16 (halves the input DMA), projections produce
   QT [2 heads stacked per 128 partitions], KT (two partition-offset
   copies so odd/even heads both get offset-matched operands) and VT.
 - V' = [V|1] built by PE-transpose (half 0) / DMA-transpose (half 1).
 - scores_T[k, q] per head per 1024-query block; exp on ScalarE in
   [128,1024] blocks writing bf16.
 - PV runs dense: h_un[q, 65] += exp_T[:, qtile].T @ V' accumulated over
   key tiles in PSUM (2x fewer PE rows than the hT-layout alternative).
 - normalize on eviction via per-partition reciprocal multiply; pairs of
   heads share an SBUF buffer that one DMA-transpose per pair flips into
   hT layout for the output projection.
 - half-1 projections and out-projection chunks are emitted as deferred
   PE tasks, one per exp slot, so the Tensor engine queue always has
   work but never starves the exp pipeline.
"""

from collections import deque

import numpy as np

NUM_HEADS = 16
Dh = 64
B, S, D = 2, 2048, 1024
G = 4            # head groups (cores per batch)
HG = 4           # heads per group
QD = HG * Dh     # 256 local q dims
NK = D // 128    # 8 contraction tiles for projections
NSK = S // 128   # 16 key tiles
W = 512          # matmul moving width
HALF = 1024      # query block / projection column half
N_CORES = 8

_CACHE = {}
_DEBUG = False


def _build_nc():
    from contextlib import ExitStack

    import concourse.bass as bass
    import concourse.mybir as mybir
    import concourse.tile as tile
    from concourse import bacc
    from concourse.masks import make_identity

    F32 = mybir.dt.float32
    BF16 = mybir.dt.bfloat16
    EXP = mybir.ActivationFunctionType.Exp

    nc = bacc.Bacc("TRN2", target_bir_lowering=False, debug=False)

    xT = nc.declare_dram_parameter("xT", [D, S], BF16, isOutput=False)
    wqT = nc.declare_dram_parameter("wqT", [D, QD], BF16, isOutput=False)
    wkvT = nc.declare_dram_parameter("wkvT", [D, 128], BF16, isOutput=False)
    woT = nc.declare_dram_parameter("woT", [QD, D], BF16, isOutput=False)
    bqp = nc.declare_dram_parameter("bq", [QD, 1], F32, isOutput=False)
    part = nc.declare_dram_parameter("part", [S, D], BF16, isOutput=True)
    if _DEBUG:
        dbg = {
            "d_qt": nc.declare_dram_parameter("d_qt", [128, 2 * S], BF16, isOutput=True),
            "d_kt2": nc.declare_dram_parameter("d_kt2", [128, S], BF16, isOutput=True),
            "d_v1": nc.declare_dram_parameter("d_v1", [128, NSK * 65], BF16, isOutput=True),
            "d_ht": nc.declare_dram_parameter("d_ht", [128, 2 * S], BF16, isOutput=True),
        }

    with tile.TileContext(nc) as tc, ExitStack() as ctx:
        const = ctx.enter_context(tc.tile_pool(name="const", bufs=1))
        persist = ctx.enter_context(tc.tile_pool(name="persist", bufs=1))

        wq_sb = const.tile([128, NK * QD], BF16)    # ktile kt at cols [kt*QD:+QD]
        wkv_sb = const.tile([128, NK * 128], BF16)  # cols 0:64 = wkT, 64:128 = wvT
        wo_sb = const.tile([128, 2 * D], BF16)      # q-ktile p at cols [p*D:+D]
        bq_sb = const.tile([128, 2], F32)
        ident = const.tile([128, 128], BF16)

        qt_sb = persist.tile([128, 2 * S], BF16)    # pt p cols [p*S:+S]; rows 0:64 head 2p, 64:128 head 2p+1
        kt2_sb = persist.tile([128, S], BF16)       # KT duplicated rows 0:64 and 64:128
        vt_sb = persist.tile([128, S], BF16)        # VT in rows 64:128
        v1_sb = persist.tile([128, NSK * 65], BF16)  # V' tile sk at cols [sk*65:+65]
        ht_sb = persist.tile([128, 2 * S], BF16)    # hT, q-ktile p at cols [p*S:+S]

        make_identity(nc, ident[:])
        nc.vector.memset(v1_sb[:], 1.0)  # pre-fill the softmax-denominator columns

        # ---- DMAs + half-0 projections --------------------------------
        # x lands in 4 tiles of 2 ktiles (half 0, fine overlap with the
        # projection chains) + 2 tiles of 4 ktiles (half 1).  Issue queues
        # are spread over SP/ACT/DVE so the ~1.2us per-DMA issue cost
        # doesn't serialize the critical path.
        xp = ctx.enter_context(tc.tile_pool(name="xp", bufs=1))
        x0 = [xp.tile([128, 2 * HALF], BF16, name=f"x0_{j}") for j in range(4)]
        x1 = [xp.tile([128, 2 * HALF], BF16, name=f"x1_{j}") for j in range(4)]

        def dma_x(eng, tiles, j, hf):
            eng.dma_start(
                tiles[j][:].rearrange("p (k c) -> p k c", c=HALF),
                xT[j * 256:(j + 1) * 256, hf * HALF:(hf + 1) * HALF].rearrange(
                    "(k p) c -> p k c", p=128),
            )

        # weights issue from the ACT queue (idle until the first exp at
        # ~12us), x tiles from SP: neither queue's serialized ~1.2-2us
        # per-DMA issue cost then gates the critical path
        nc.scalar.dma_start(
            wkv_sb[:].rearrange("p (k c) -> p k c", c=128),
            wkvT[:, :].rearrange("(k p) c -> p k c", p=128),
        )
        nc.scalar.dma_start(
            wq_sb[:].rearrange("p (k c) -> p k c", c=QD),
            wqT[:, :].rearrange("(k p) c -> p k c", p=128),
        )
        for j in range(4):
            dma_x(nc.sync, x0, j, 0)
        for j in range(4):
            dma_x(nc.sync, x1, j, 1)
        for p in range(2):
            nc.scalar.dma_start(bq_sb[:, p:p + 1], bqp[p * 128:(p + 1) * 128, :])
        nc.scalar.dma_start(
            wo_sb[:].rearrange("p (a c) -> p a c", c=D),
            woT[:, :].rearrange("(a p) c -> p a c", p=128),
        )

        def xcol(kt, hf, n):
            tiles = x0 if hf == 0 else x1
            return tiles[kt // 2][:, (kt % 2) * HALF + n * W:(kt % 2) * HALF + (n + 1) * W]

        def proj_kt_step(ps, wsb_col, kt, hf, width):
            for n in range(HALF // W):
                nc.tensor.matmul(
                    ps[:, n * W:(n + 1) * W],
                    lhsT=wsb_col(kt),
                    rhs=xcol(kt, hf, n),
                    start=(kt == 0), stop=(kt == NK - 1),
                )

        wkv_col = lambda kt: wkv_sb[:, kt * 128:(kt + 1) * 128]
        wq0_col = lambda kt: wq_sb[:, kt * QD:kt * QD + 128]
        wq1_col = lambda kt: wq_sb[:, kt * QD + 128:kt * QD + 256]

        with (
            tc.tile_pool(name="p0", bufs=1, space="PSUM") as p0,
            tc.tile_pool(name="trps", bufs=2, space="PSUM") as trps,
        ):
            vk_ps = p0.tile([128, HALF], F32, name="vk")
            q0_ps = p0.tile([128, HALF], F32, name="q0")
            q1_ps = p0.tile([128, HALF], F32, name="q1")
            # vk + q0 gate the first exp; q1 (head 1) can trail.
            for kt in range(NK):
                proj_kt_step(vk_ps, wkv_col, kt, 0, HALF)
                proj_kt_step(q0_ps, wq0_col, kt, 0, HALF)
            nc.scalar.copy(kt2_sb[0:64, 0:HALF], vk_ps[0:64, :])
            nc.vector.tensor_scalar_add(qt_sb[:, 0:HALF], q0_ps[:], bq_sb[:, 0:1])
            for kt in range(NK):
                proj_kt_step(q1_ps, wq1_col, kt, 0, HALF)
            nc.vector.tensor_copy(vt_sb[64:128, 0:HALF], vk_ps[64:128, :])
            nc.gpsimd.tensor_copy(kt2_sb[64:128, 0:HALF], kt2_sb[0:64, 0:HALF])
            nc.vector.tensor_scalar_add(qt_sb[:, S:S + HALF], q1_ps[:], bq_sb[:, 1:2])
            # V' half 0 by PE transpose (DMA engines are busy with x half 1)
            for sk in range(NSK // 2):
                tr = trps.tile([128, Dh], BF16, name="tr")
                nc.tensor.transpose(
                    tr[:], vt_sb[64:128, sk * 128:(sk + 1) * 128],
                    ident[64:128, 64:128],
                )
                nc.vector.tensor_copy(v1_sb[:, sk * 65:sk * 65 + 64], tr[:])

        # ---- Phase 2: attention, with deferred fill-in PE tasks -------
        expp = ctx.enter_context(tc.tile_pool(name="expp", bufs=8))
        hpp = ctx.enter_context(tc.tile_pool(name="hpp", bufs=2))
        smalls = ctx.enter_context(tc.tile_pool(name="smalls", bufs=4))
        osbp = ctx.enter_context(tc.tile_pool(name="osbp", bufs=2))
        # scores/h_un PSUM lives in its own stack so the final-block flush
        # can reclaim those banks for a deeper out-projection pipeline
        scph_stack = ExitStack()
        scps = scph_stack.enter_context(tc.tile_pool(name="scps", bufs=2, space="PSUM"))
        hups = scph_stack.enter_context(tc.tile_pool(name="hups", bufs=1, space="PSUM"))

        state = {}

        def mk_vk1(kts):
            def t():
                if "vk1" not in state:
                    pvk = pvk_stack.enter_context(
                        tc.tile_pool(name="pvk", bufs=1, space="PSUM"))
                    state["vk1"] = pvk.tile([128, HALF], F32, name="vkps")
                for kt in kts:
                    proj_kt_step(state["vk1"], wkv_col, kt, 1, HALF)
            return t

        def t_kv1_evict():
            vk1 = state.pop("vk1")
            nc.vector.tensor_copy(kt2_sb[0:64, HALF:S], vk1[0:64, :])
            nc.vector.tensor_copy(vt_sb[64:128, HALF:S], vk1[64:128, :])
            nc.gpsimd.tensor_copy(kt2_sb[64:128, HALF:S], kt2_sb[0:64, HALF:S])
            pvk_stack.close()

        def t_v1_h1():
            # V' half 1 by PE transpose, in a short-lived 1-bank PSUM era
            # (dma_start_transpose mis-executes the 65-strided pattern on HW)
            with tc.tile_pool(name="trps1", bufs=2, space="PSUM") as trps1:
                for sk in range(NSK // 2, NSK):
                    tr = trps1.tile([128, Dh], BF16, name="tr1")
                    nc.tensor.transpose(
                        tr[:], vt_sb[64:128, sk * 128:(sk + 1) * 128],
                        ident[64:128, 64:128],
                    )
                    nc.vector.tensor_copy(v1_sb[:, sk * 65:sk * 65 + 64], tr[:])

        def mk_q1(which, col_fn, kts):
            def t():
                if which in state:
                    ps = state[which]
                else:
                    if "pq" not in state:
                        state["pq"] = pq_stack.enter_context(
                            tc.tile_pool(name="pq", bufs=1, space="PSUM"))
                    ps = state[which] = state["pq"].tile([128, HALF], F32, name="qps")
                for kt in kts:
                    proj_kt_step(ps, col_fn, kt, 1, HALF)
            return t

        def mk_q1_evict(which, pt):
            def t():
                ps = state.pop(which)
                nc.vector.tensor_scalar_add(
                    qt_sb[:, pt * S + HALF:pt * S + S], ps[:], bq_sb[:, pt:pt + 1]
                )
            return t

        def t_close_p1b():
            pq_stack.close()

        tasks = deque(
            [None,
             mk_vk1(range(0, 2)), mk_vk1(range(2, 4)),
             mk_vk1(range(4, 6)), mk_vk1(range(6, 8)),
             t_kv1_evict, t_v1_h1]
            + [mk_q1("q0h1", wq0_col, range(kt, kt + 1)) for kt in range(NK)]
            + [mk_q1_evict("q0h1", 0)]
            + [mk_q1("q1h1", wq1_col, range(kt, kt + 1)) for kt in range(NK)]
            + [mk_q1_evict("q1h1", 1), t_close_p1b]
        )

        outps_stack = ExitStack()
        outps = None
        pvk_stack = ExitStack()
        pq_stack = ExitStack()

        def mk_outproj(sc_i, n, p):
            # block-0 out-projection fill-in: one matmul per task (~213ns)
            # so it never blows the per-exp-slot PE budget
            def t():
                nonlocal outps
                if outps is None:
                    outps = outps_stack.enter_context(
                        tc.tile_pool(name="outps", bufs=2, space="PSUM"))
                s = sc_i
                half = sc_i % 2
                if half == 0 and n == 0 and p == 0:
                    state["osb"] = osbp.tile([128, 2 * D], BF16, name="osb")
                o_sb = state["osb"]
                if p == 0:
                    state["ops"] = outps.tile([128, W], F32, name="ops")
                o_ps = state["ops"]
                nc.tensor.matmul(
                    o_ps[:],
                    lhsT=ht_sb[:, p * S + s * 128:p * S + (s + 1) * 128],
                    rhs=wo_sb[:, p * D + n * W:p * D + (n + 1) * W],
                    start=(p == 0), stop=(p == 1),
                )
                if p == 1:
                    nc.vector.tensor_copy(
                        o_sb[:, half * D + n * W:half * D + (n + 1) * W], o_ps[:])
                if half == 1 and n == 1 and p == 1:
                    nc.sync.dma_start(
                        part[(s - 1) * 128:(s + 1) * 128, :].rearrange(
                            "(c p) d -> p c d", p=128),
                        o_sb[:].rearrange("p (c d) -> p c d", d=D),
                    )
                    if s == (HALF // 128) - 1:
                        outps_stack.close()
            return t

        hp = None
        for bI in range(2):
            for h in range(HG):
                pt, hi = h // 2, h % 2
                qbase = pt * S + bI * HALF
                if hi == 0:
                    hp = hpp.tile([128, HALF], BF16, name="hp")
                hu = [hups.tile([128, 260], F32, name=f"hu{g}") for g in range(2)]
                ets = [None] * NSK

                def emit_pv(sk, hu=hu, ets=ets):
                    for qt in range(8):
                        nc.tensor.matmul(
                            hu[qt // 4][:, (qt % 4) * 65:(qt % 4) * 65 + 65],
                            lhsT=ets[sk][:, qt * 128:(qt + 1) * 128],
                            rhs=v1_sb[:, sk * 65:(sk + 1) * 65],
                            # start zeroes the whole 2KB PSUM bank (all 4 qt
                            # regions), so only the bank's first/last matmul
                            # opens/closes the accumulation group
                            start=(sk == 0 and qt % 4 == 0),
                            stop=(sk == NSK - 1 and qt % 4 == 3),
                        )

                for sk in range(NSK):
                    # PV trails exp by 4 slots: the first PV of a head waits
                    # on the previous head's norm-eviction freeing the hu
                    # banks, and a deeper pipeline keeps that wait off the
                    # in-order PE queue's critical path.
                    if sk >= 4:
                        emit_pv(sk - 4)
                    sc = scps.tile([128, HALF], F32, name="sc")
                    for n in range(2):
                        nc.tensor.matmul(
                            sc[:, n * W:(n + 1) * W],
                            lhsT=kt2_sb[hi * 64:(hi + 1) * 64, sk * 128:(sk + 1) * 128],
                            rhs=qt_sb[hi * 64:(hi + 1) * 64, qbase + n * W:qbase + (n + 1) * W],
                            start=True, stop=True,
                        )
                    et = expp.tile([128, HALF], BF16, name="et")
                    nc.scalar.activation(et[:], sc[:], EXP, scale=0.125)
                    ets[sk] = et
                    if tasks:
                        t = tasks.popleft()
                        if t is not None:
                            t()
                for sk in range(NSK - 4, NSK):
                    emit_pv(sk)

                # normalize on eviction: hp[q, hi*64+d] = h_un[q, d] / sumexp[q]
                rec = smalls.tile([128, 8], F32, name="rec")
                for g in range(2):
                    for q4 in range(4):
                        nc.vector.reciprocal(
                            rec[:, g * 4 + q4:g * 4 + q4 + 1],
                            hu[g][:, q4 * 65 + 64:q4 * 65 + 65],
                        )
                for qt in range(8):
                    nc.vector.tensor_scalar_mul(
                        hp[:, qt * 128 + hi * 64:qt * 128 + hi * 64 + 64],
                        hu[qt // 4][:, (qt % 4) * 65:(qt % 4) * 65 + 64],
                        rec[:, qt:qt + 1],
                    )
                    if hi == 1 and qt in (3, 7):
                        # transpose each 512-col half as soon as its four
                        # norm-evictions land: halves the transpose latency
                        # hanging off the last head's tail
                        lo = (qt - 3) * 128
                        dst = ht_sb[:, pt * S + bI * HALF + lo:
                                    pt * S + bI * HALF + lo + 4 * 128]
                        dst = dst.rearrange("p (a b) -> p a b", b=128)
                        nc.sync.dma_start_transpose(dst, hp[:, lo:lo + 4 * 128])
                if hi == 1 and pt == 1 and bI == 0:
                    for sc_i in range(HALF // 128):
                        for n in range(2):
                            for p in range(2):
                                tasks.append(mk_outproj(sc_i, n, p))
        while tasks:
            t = tasks.popleft()
            if t is not None:
                t()
        if outps is not None:
            outps_stack.close()

        # ---- final flush: block-1 out-projection with the reclaimed
        # scores/h_un banks giving a 4-deep PSUM pipeline ----------------
        scph_stack.close()
        with tc.tile_pool(name="outpsB", bufs=4, space="PSUM") as outpsB:
            for sc_i in range(HALF // 128):
                s = (HALF // 128) + sc_i
                o_sb = osbp.tile([128, D], BF16, name="osbB")
                for n in range(2):
                    o_ps = outpsB.tile([128, W], F32, name="opsB")
                    for p in range(2):
                        nc.tensor.matmul(
                            o_ps[:],
                            lhsT=ht_sb[:, p * S + s * 128:p * S + (s + 1) * 128],
                            rhs=wo_sb[:, p * D + n * W:p * D + (n + 1) * W],
                            start=(p == 0), stop=(p == 1),
                        )
                    dst = o_sb[:, n * W:(n + 1) * W]
                    # 2:1 DVE:ACT eviction split keeps both engines under
                    # the ~854ns/chunk PE cadence
                    if n == 1 and sc_i % 3 != 2:
                        nc.scalar.copy(dst, o_ps[:])
                    else:
                        nc.vector.tensor_copy(dst, o_ps[:])
                nc.sync.dma_start(part[s * 128:(s + 1) * 128, :], o_sb[:])
        if _DEBUG:
            nc.sync.dma_start(dbg["d_qt"][:, :], qt_sb[:])
            nc.sync.dma_start(dbg["d_kt2"][:, :], kt2_sb[:])
            nc.sync.dma_start(dbg["d_v1"][:, :], v1_sb[:])
            nc.sync.dma_start(dbg["d_ht"][:, :], ht_sb[:])

    nc.finalize()
    return nc


def _get_nc():
    if "nc" not in _CACHE:
        _CACHE["nc"] = _build_nc()
    return _CACHE["nc"]


def _prep_core_inputs(inputs, wq, bq, wk, wv, wo):
    """Host-side shard prep: per-core transposed/rearranged bf16 operands."""
    from ml_dtypes import bfloat16

    xT = [np.ascontiguousarray(np.asarray(inputs[b], np.float32).T).astype(bfloat16)
          for b in range(B)]
    wq3 = np.asarray(wq, np.float32).reshape(Dh, NUM_HEADS, D)
    bq2 = np.asarray(bq, np.float32).reshape(Dh, NUM_HEADS)
    wkvT = np.ascontiguousarray(
        np.concatenate([np.asarray(wk, np.float32).T, np.asarray(wv, np.float32).T],
                       axis=1)
    ).astype(bfloat16)  # [1024, 128], K in cols 0:64
    wo_ = np.asarray(wo, np.float32)

    in_maps = []
    for c in range(N_CORES):
        b, g = divmod(c, G)
        heads = [g * HG + hl for hl in range(HG)]
        wqT_g = np.ascontiguousarray(
            np.concatenate([wq3[:, h, :].T for h in heads], axis=1)
        ).astype(bfloat16)
        bq_g = np.ascontiguousarray(
            np.concatenate([bq2[:, h] for h in heads]).reshape(QD, 1).astype(np.float32)
        )
        woT_g = np.ascontiguousarray(
            wo_[:, g * QD:(g + 1) * QD].T
        ).astype(bfloat16)  # [256, 1024]
        in_maps.append({
            "xT": xT[b],
            "wqT": wqT_g,
            "wkvT": wkvT,
            "woT": woT_g,
            "bq": bq_g,
        })
    return in_maps


def kernel(inputs, wq, bq, wk, bk, wv, bv, wo, bo):
    from concourse.bass_utils import run_bass_kernel_spmd

    nc = _get_nc()
    in_maps = _prep_core_inputs(inputs, wq, bq, wk, wv, wo)
    res = run_bass_kernel_spmd(nc, in_maps, list(range(N_CORES))).results

    wo_ = np.asarray(wo, np.float32)
    bias = (
        np.asarray(bo, np.float32)
        + wo_ @ np.tile(np.asarray(bv, np.float32), NUM_HEADS)
    )
    out = np.empty((B, S, D), np.float32)
    for b in range(B):
        acc = res[b * G]["part"].astype(np.float32).copy()
        for g in range(1, G):
            acc += res[b * G + g]["part"]
        out[b] = acc + bias
    return out


# revision 42
# speedup vs baseline: 1.3041x; 1.0415x over previous
"""MQA self-attention kernel for Trainium2, 8 NeuronCores.

Reference computation (fp32):
    q = x @ wq.T + bq        -> [B,S,1024] -> heads via (hidden num_heads) split
    k = x @ wk.T + bk        -> [B,S,64]  (single shared KV head)
    v = x @ wv.T + bv
    scores = q @ k.T / 8 ; attn = softmax(scores) ; h = attn @ v
    out = merge_heads(h) @ wo.T + bo

Sharding (8 cores, no collectives): core c handles batch b=c//4 and head
group g=c%4 (4 of the 16 q-heads).  The shared K/V head is replicated.
Each core returns the partial output h_g @ wo_g.T [S, D]; the host sums
the 4 head-group partials per batch and adds the bias terms.

Math notes:
 - bk provably cancels in softmax; bv is folded into the host-side output
   bias (softmax rows sum to 1); softmax runs without max subtraction
   (scores ~ N(0,1), exp stays within bf16/f32 range).

Device pipeline (all operands bf16, PSUM f32; the Activation engine's
exp throughput ~133us/core is the hard floor, so every other engine is
kept strictly below it):
 - xT/weights land as bf16 (halves the input DMA), projections produce
   QT [2 heads stacked per 128 partitions], KT (two partition-offset
   copies so odd/even heads both get offset-matched operands) and VT.
 - V' = [V|1] built by PE-transpose (half 0) / DMA-transpose (half 1).
 - scores_T[k, q] per head per 1024-query block; exp on ScalarE in
   [128,1024] blocks writing bf16.
 - PV runs dense: h_un[q, 65] += exp_T[:, qtile].T @ V' accumulated over
   key tiles in PSUM (2x fewer PE rows than the hT-layout alternative).
 - normalize on eviction via per-partition reciprocal multiply; pairs of
   heads share an SBUF buffer that one DMA-transpose per pair flips into
   hT layout for the output projection.
 - half-1 projections and out-projection chunks are emitted as deferred
   PE tasks, one per exp slot, so the Tensor engine queue always has
   work but never starves the exp pipeline.
"""

from collections import deque

import numpy as np

NUM_HEADS = 16
Dh = 64
B, S, D = 2, 2048, 1024
G = 4            # head groups (cores per batch)
HG = 4           # heads per group
QD = HG * Dh     # 256 local q dims
NK = D // 128    # 8 contraction tiles for projections
NSK = S // 128   # 16 key tiles
W = 512          # matmul moving width
HALF = 1024      # query block / projection column half
N_CORES = 8

_CACHE = {}
_DEBUG = False


def _build_nc():
    from contextlib import ExitStack

    import concourse.bass as bass
    import concourse.mybir as mybir
    import concourse.tile as tile
    from concourse import bacc
    from concourse.masks import make_identity

    F32 = mybir.dt.float32
    BF16 = mybir.dt.bfloat16
    EXP = mybir.ActivationFunctionType.Exp

    nc = bacc.Bacc("TRN2", target_bir_lowering=False, debug=False)

    xT = nc.declare_dram_parameter("xT", [D, S], BF16, isOutput=False)
    wqT = nc.declare_dram_parameter("wqT", [D, QD], BF16, isOutput=False)
    wkvT = nc.declare_dram_parameter("wkvT", [D, 128], BF16, isOutput=False)
    woT = nc.declare_dram_parameter("woT", [QD, D], BF16, isOutput=False)
    bqp = nc.declare_dram_parameter("bq", [QD, 1], F32, isOutput=False)
    part = nc.declare_dram_parameter("part", [S, D], BF16, isOutput=True)
    if _DEBUG:
        dbg = {
            "d_qt": nc.declare_dram_parameter("d_qt", [128, 2 * S], BF16, isOutput=True),
            "d_kt2": nc.declare_dram_parameter("d_kt2", [128, S], BF16, isOutput=True),
            "d_v1": nc.declare_dram_parameter("d_v1", [128, NSK * 65], BF16, isOutput=True),
            "d_ht": nc.declare_dram_parameter("d_ht", [128, 2 * S], BF16, isOutput=True),
        }

    with tile.TileContext(nc) as tc, ExitStack() as ctx:
        const = ctx.enter_context(tc.tile_pool(name="const", bufs=1))
        persist = ctx.enter_context(tc.tile_pool(name="persist", bufs=1))

        wq_sb = const.tile([128, NK * QD], BF16)    # ktile kt at cols [kt*QD:+QD]
        wkv_sb = const.tile([128, NK * 128], BF16)  # cols 0:64 = wkT, 64:128 = wvT
        wo_sb = const.tile([128, 2 * D], BF16)      # q-ktile p at cols [p*D:+D]
        bq_sb = const.tile([128, 2], F32)
        ident = const.tile([128, 128], BF16)

        qt_sb = persist.tile([128, 2 * S], BF16)    # pt p cols [p*S:+S]; rows 0:64 head 2p, 64:128 head 2p+1
        kt2_sb = persist.tile([128, S], BF16)       # KT duplicated rows 0:64 and 64:128
        vt_sb = persist.tile([128, S], BF16)        # VT in rows 64:128
        v1_sb = persist.tile([128, NSK * 65], BF16)  # V' tile sk at cols [sk*65:+65]
        ht_sb = persist.tile([128, 2 * S], BF16)    # hT, q-ktile p at cols [p*S:+S]

        make_identity(nc, ident[:])
        nc.vector.memset(v1_sb[:], 1.0)  # pre-fill the softmax-denominator columns

        # ---- DMAs + half-0 projections --------------------------------
        # x lands in 4 tiles of 2 ktiles (half 0, fine overlap with the
        # projection chains) + 2 tiles of 4 ktiles (half 1).  Issue queues
        # are spread over SP/ACT/DVE so the ~1.2us per-DMA issue cost
        # doesn't serialize the critical path.
        xp = ctx.enter_context(tc.tile_pool(name="xp", bufs=1))
        x0 = [xp.tile([128, 2 * HALF], BF16, name=f"x0_{j}") for j in range(4)]
        x1 = [xp.tile([128, 2 * HALF], BF16, name=f"x1_{j}") for j in range(4)]

        def dma_x(eng, tiles, j, hf):
            eng.dma_start(
                tiles[j][:].rearrange("p (k c) -> p k c", c=HALF),
                xT[j * 256:(j + 1) * 256, hf * HALF:(hf + 1) * HALF].rearrange(
                    "(k p) c -> p k c", p=128),
            )

        # weights issue from the ACT queue (idle until the first exp at
        # ~12us), x tiles from SP: neither queue's serialized ~1.2-2us
        # per-DMA issue cost then gates the critical path
        nc.scalar.dma_start(
            wkv_sb[:].rearrange("p (k c) -> p k c", c=128),
            wkvT[:, :].rearrange("(k p) c -> p k c", p=128),
        )
        nc.scalar.dma_start(
            wq_sb[:].rearrange("p (k c) -> p k c", c=QD),
            wqT[:, :].rearrange("(k p) c -> p k c", p=128),
        )
        for j in range(4):
            dma_x(nc.sync, x0, j, 0)
        for j in range(4):
            dma_x(nc.sync, x1, j, 1)
        for p in range(2):
            nc.scalar.dma_start(bq_sb[:, p:p + 1], bqp[p * 128:(p + 1) * 128, :])
        nc.scalar.dma_start(
            wo_sb[:].rearrange("p (a c) -> p a c", c=D),
            woT[:, :].rearrange("(a p) c -> p a c", p=128),
        )

        def xcol(kt, hf, n):
            tiles = x0 if hf == 0 else x1
            return tiles[kt // 2][:, (kt % 2) * HALF + n * W:(kt % 2) * HALF + (n + 1) * W]

        def proj_kt_step(ps, wsb_col, kt, hf, width):
            for n in range(HALF // W):
                nc.tensor.matmul(
                    ps[:, n * W:(n + 1) * W],
                    lhsT=wsb_col(kt),
                    rhs=xcol(kt, hf, n),
                    start=(kt == 0), stop=(kt == NK - 1),
                )

        wkv_col = lambda kt: wkv_sb[:, kt * 128:(kt + 1) * 128]
        wq0_col = lambda kt: wq_sb[:, kt * QD:kt * QD + 128]
        wq1_col = lambda kt: wq_sb[:, kt * QD + 128:kt * QD + 256]

        with (
            tc.tile_pool(name="p0", bufs=1, space="PSUM") as p0,
            tc.tile_pool(name="trps", bufs=2, space="PSUM") as trps,
        ):
            vk_ps = p0.tile([128, HALF], F32, name="vk")
            q0_ps = p0.tile([128, HALF], F32, name="q0")
            # vk + q0 gate the first exp; q1 (head 1) runs as deferred
            # fill-in so the scheduler can't wedge it between early scores
            for kt in range(NK):
                proj_kt_step(vk_ps, wkv_col, kt, 0, HALF)
                proj_kt_step(q0_ps, wq0_col, kt, 0, HALF)
            nc.scalar.copy(kt2_sb[0:64, 0:HALF], vk_ps[0:64, :])
            nc.vector.tensor_scalar_add(qt_sb[:, 0:HALF], q0_ps[:], bq_sb[:, 0:1])
            nc.vector.tensor_copy(vt_sb[64:128, 0:HALF], vk_ps[64:128, :])
            nc.gpsimd.tensor_copy(kt2_sb[64:128, 0:HALF], kt2_sb[0:64, 0:HALF])
            # V' half 0 by PE transpose (DMA engines are busy with x half 1)
            for sk in range(NSK // 2):
                tr = trps.tile([128, Dh], BF16, name="tr")
                nc.tensor.transpose(
                    tr[:], vt_sb[64:128, sk * 128:(sk + 1) * 128],
                    ident[64:128, 64:128],
                )
                nc.vector.tensor_copy(v1_sb[:, sk * 65:sk * 65 + 64], tr[:])

        # ---- Phase 2: attention, with deferred fill-in PE tasks -------
        expp = ctx.enter_context(tc.tile_pool(name="expp", bufs=8))
        hpp = ctx.enter_context(tc.tile_pool(name="hpp", bufs=2))
        smalls = ctx.enter_context(tc.tile_pool(name="smalls", bufs=4))
        osbp = ctx.enter_context(tc.tile_pool(name="osbp", bufs=2))
        # scores/h_un PSUM lives in its own stack so the final-block flush
        # can reclaim those banks for a deeper out-projection pipeline
        scph_stack = ExitStack()
        scps = scph_stack.enter_context(tc.tile_pool(name="scps", bufs=2, space="PSUM"))
        hups = scph_stack.enter_context(tc.tile_pool(name="hups", bufs=1, space="PSUM"))

        state = {}

        def mk_vk1(kts):
            def t():
                if "vk1" not in state:
                    pvk = pvk_stack.enter_context(
                        tc.tile_pool(name="pvk", bufs=1, space="PSUM"))
                    state["vk1"] = pvk.tile([128, HALF], F32, name="vkps")
                for kt in kts:
                    proj_kt_step(state["vk1"], wkv_col, kt, 1, HALF)
            return t

        def t_kv1_evict():
            vk1 = state.pop("vk1")
            nc.vector.tensor_copy(kt2_sb[0:64, HALF:S], vk1[0:64, :])
            nc.vector.tensor_copy(vt_sb[64:128, HALF:S], vk1[64:128, :])
            nc.gpsimd.tensor_copy(kt2_sb[64:128, HALF:S], kt2_sb[0:64, HALF:S])
            pvk_stack.close()

        def t_v1_h1():
            # V' half 1 by PE transpose, in a short-lived 1-bank PSUM era
            # (dma_start_transpose mis-executes the 65-strided pattern on HW)
            with tc.tile_pool(name="trps1", bufs=2, space="PSUM") as trps1:
                for sk in range(NSK // 2, NSK):
                    tr = trps1.tile([128, Dh], BF16, name="tr1")
                    nc.tensor.transpose(
                        tr[:], vt_sb[64:128, sk * 128:(sk + 1) * 128],
                        ident[64:128, 64:128],
                    )
                    nc.vector.tensor_copy(v1_sb[:, sk * 65:sk * 65 + 64], tr[:])

        def mk_q1(which, col_fn, kts, hf):
            def t():
                if which in state:
                    ps = state[which]
                else:
                    if "pq" not in state:
                        state["pq"] = pq_stack.enter_context(
                            tc.tile_pool(name="pq", bufs=1, space="PSUM"))
                    ps = state[which] = state["pq"].tile([128, HALF], F32, name="qps")
                for kt in kts:
                    proj_kt_step(ps, col_fn, kt, hf, HALF)
            return t

        def mk_q1_evict(which, pt, hf):
            def t():
                ps = state.pop(which)
                c0 = pt * S + hf * HALF
                nc.vector.tensor_scalar_add(
                    qt_sb[:, c0:c0 + HALF], ps[:], bq_sb[:, pt:pt + 1]
                )
            return t

        def t_close_p1b():
            pq_stack.close()

        tasks = deque(
            [None,
             mk_vk1(range(0, 2)), mk_vk1(range(2, 4)),
             mk_vk1(range(4, 6)), mk_vk1(range(6, 8)),
             t_kv1_evict, t_v1_h1]
            + [mk_q1("q1h0", wq1_col, range(kt, kt + 2), 0) for kt in range(0, NK, 2)]
            + [mk_q1_evict("q1h0", 1, 0)]
            + [mk_q1("q0h1", wq0_col, range(kt, kt + 1), 1) for kt in range(NK)]
            + [mk_q1_evict("q0h1", 0, 1)]
            + [mk_q1("q1h1", wq1_col, range(kt, kt + 1), 1) for kt in range(NK)]
            + [mk_q1_evict("q1h1", 1, 1), t_close_p1b]
        )

        outps_stack = ExitStack()
        outps = None
        pvk_stack = ExitStack()
        pq_stack = ExitStack()

        def mk_outproj(sc_i, n, p):
            # block-0 out-projection fill-in: one matmul per task (~213ns)
            # so it never blows the per-exp-slot PE budget
            def t():
                nonlocal outps
                if outps is None:
                    outps = outps_stack.enter_context(
                        tc.tile_pool(name="outps", bufs=2, space="PSUM"))
                s = sc_i
                half = sc_i % 2
                if half == 0 and n == 0 and p == 0:
                    state["osb"] = osbp.tile([128, 2 * D], BF16, name="osb")
                o_sb = state["osb"]
                if p == 0:
                    state["ops"] = outps.tile([128, W], F32, name="ops")
                o_ps = state["ops"]
                nc.tensor.matmul(
                    o_ps[:],
                    lhsT=ht_sb[:, p * S + s * 128:p * S + (s + 1) * 128],
                    rhs=wo_sb[:, p * D + n * W:p * D + (n + 1) * W],
                    start=(p == 0), stop=(p == 1),
                )
                if p == 1:
                    nc.vector.tensor_copy(
                        o_sb[:, half * D + n * W:half * D + (n + 1) * W], o_ps[:])
                if half == 1 and n == 1 and p == 1:
                    nc.sync.dma_start(
                        part[(s - 1) * 128:(s + 1) * 128, :].rearrange(
                            "(c p) d -> p c d", p=128),
                        o_sb[:].rearrange("p (c d) -> p c d", d=D),
                    )
                    if s == (HALF // 128) - 1:
                        outps_stack.close()
            return t

        hp = None
        for bI in range(2):
            for h in range(HG):
                pt, hi = h // 2, h % 2
                qbase = pt * S + bI * HALF
                if hi == 0:
                    hp = hpp.tile([128, HALF], BF16, name="hp")
                hu = [hups.tile([128, 260], F32, name=f"hu{g}") for g in range(2)]
                ets = [None] * NSK

                def emit_pv(sk, hu=hu, ets=ets):
                    for qt in range(8):
                        nc.tensor.matmul(
                            hu[qt // 4][:, (qt % 4) * 65:(qt % 4) * 65 + 65],
                            lhsT=ets[sk][:, qt * 128:(qt + 1) * 128],
                            rhs=v1_sb[:, sk * 65:(sk + 1) * 65],
                            # start zeroes the whole 2KB PSUM bank (all 4 qt
                            # regions), so only the bank's first/last matmul
                            # opens/closes the accumulation group
                            start=(sk == 0 and qt % 4 == 0),
                            stop=(sk == NSK - 1 and qt % 4 == 3),
                        )

                for sk in range(NSK):
                    # PV trails exp by 4 slots: the first PV of a head waits
                    # on the previous head's norm-eviction freeing the hu
                    # banks, and a deeper pipeline keeps that wait off the
                    # in-order PE queue's critical path.
                    if sk >= 4:
                        emit_pv(sk - 4)
                    sc = scps.tile([128, HALF], F32, name="sc")
                    for n in range(2):
                        nc.tensor.matmul(
                            sc[:, n * W:(n + 1) * W],
                            lhsT=kt2_sb[hi * 64:(hi + 1) * 64, sk * 128:(sk + 1) * 128],
                            rhs=qt_sb[hi * 64:(hi + 1) * 64, qbase + n * W:qbase + (n + 1) * W],
                            start=True, stop=True,
                        )
                    et = expp.tile([128, HALF], BF16, name="et")
                    nc.scalar.activation(et[:], sc[:], EXP, scale=0.125)
                    ets[sk] = et
                    if tasks:
                        t = tasks.popleft()
                        if t is not None:
                            t()
                for sk in range(NSK - 4, NSK):
                    emit_pv(sk)

                # normalize on eviction: hp[q, hi*64+d] = h_un[q, d] / sumexp[q]
                rec = smalls.tile([128, 8], F32, name="rec")
                for g in range(2):
                    for q4 in range(4):
                        nc.vector.reciprocal(
                            rec[:, g * 4 + q4:g * 4 + q4 + 1],
                            hu[g][:, q4 * 65 + 64:q4 * 65 + 65],
                        )
                for qt in range(8):
                    nc.vector.tensor_scalar_mul(
                        hp[:, qt * 128 + hi * 64:qt * 128 + hi * 64 + 64],
                        hu[qt // 4][:, (qt % 4) * 65:(qt % 4) * 65 + 64],
                        rec[:, qt:qt + 1],
                    )
                    if hi == 1 and qt in (3, 7):
                        # transpose each 512-col half as soon as its four
                        # norm-evictions land: halves the transpose latency
                        # hanging off the last head's tail
                        lo = (qt - 3) * 128
                        dst = ht_sb[:, pt * S + bI * HALF + lo:
                                    pt * S + bI * HALF + lo + 4 * 128]
                        dst = dst.rearrange("p (a b) -> p a b", b=128)
                        nc.sync.dma_start_transpose(dst, hp[:, lo:lo + 4 * 128])
                if hi == 1 and pt == 1 and bI == 0:
                    for sc_i in range(HALF // 128):
                        for n in range(2):
                            for p in range(2):
                                tasks.append(mk_outproj(sc_i, n, p))
        while tasks:
            t = tasks.popleft()
            if t is not None:
                t()
        if outps is not None:
            outps_stack.close()

        # ---- final flush: block-1 out-projection with the reclaimed
        # scores/h_un banks giving a 4-deep PSUM pipeline ----------------
        scph_stack.close()
        with tc.tile_pool(name="outpsB", bufs=4, space="PSUM") as outpsB:
            for sc_i in range(HALF // 128):
                s = (HALF // 128) + sc_i
                half = sc_i % 2
                if half == 0:
                    state["osbB"] = osbp.tile([128, 2 * D], BF16, name="osbB")
                o_sb = state["osbB"]
                for n in range(2):
                    o_ps = outpsB.tile([128, W], F32, name="opsB")
                    for p in range(2):
                        nc.tensor.matmul(
                            o_ps[:],
                            lhsT=ht_sb[:, p * S + s * 128:p * S + (s + 1) * 128],
                            rhs=wo_sb[:, p * D + n * W:p * D + (n + 1) * W],
                            start=(p == 0), stop=(p == 1),
                        )
                    dst = o_sb[:, half * D + n * W:half * D + (n + 1) * W]
                    # 2:1 DVE:ACT eviction split keeps both engines under
                    # the ~854ns/chunk PE cadence
                    if n == 1 and sc_i % 3 != 2:
                        nc.scalar.copy(dst, o_ps[:])
                    else:
                        nc.vector.tensor_copy(dst, o_ps[:])
                if half == 1:
                    nc.sync.dma_start(
                        part[(s - 1) * 128:(s + 1) * 128, :].rearrange(
                            "(c p) d -> p c d", p=128),
                        o_sb[:].rearrange("p (c d) -> p c d", d=D),
                    )
        if _DEBUG:
            nc.sync.dma_start(dbg["d_qt"][:, :], qt_sb[:])
            nc.sync.dma_start(dbg["d_kt2"][:, :], kt2_sb[:])
            nc.sync.dma_start(dbg["d_v1"][:, :], v1_sb[:])
            nc.sync.dma_start(dbg["d_ht"][:, :], ht_sb[:])

    nc.finalize()
    return nc


def _get_nc():
    if "nc" not in _CACHE:
        _CACHE["nc"] = _build_nc()
    return _CACHE["nc"]


def _prep_core_inputs(inputs, wq, bq, wk, wv, wo):
    """Host-side shard prep: per-core transposed/rearranged bf16 operands."""
    from ml_dtypes import bfloat16

    xT = [np.ascontiguousarray(np.asarray(inputs[b], np.float32).T).astype(bfloat16)
          for b in range(B)]
    wq3 = np.asarray(wq, np.float32).reshape(Dh, NUM_HEADS, D)
    bq2 = np.asarray(bq, np.float32).reshape(Dh, NUM_HEADS)
    wkvT = np.ascontiguousarray(
        np.concatenate([np.asarray(wk, np.float32).T, np.asarray(wv, np.float32).T],
                       axis=1)
    ).astype(bfloat16)  # [1024, 128], K in cols 0:64
    wo_ = np.asarray(wo, np.float32)

    in_maps = []
    for c in range(N_CORES):
        b, g = divmod(c, G)
        heads = [g * HG + hl for hl in range(HG)]
        wqT_g = np.ascontiguousarray(
            np.concatenate([wq3[:, h, :].T for h in heads], axis=1)
        ).astype(bfloat16)
        bq_g = np.ascontiguousarray(
            np.concatenate([bq2[:, h] for h in heads]).reshape(QD, 1).astype(np.float32)
        )
        woT_g = np.ascontiguousarray(
            wo_[:, g * QD:(g + 1) * QD].T
        ).astype(bfloat16)  # [256, 1024]
        in_maps.append({
            "xT": xT[b],
            "wqT": wqT_g,
            "wkvT": wkvT,
            "woT": woT_g,
            "bq": bq_g,
        })
    return in_maps


def kernel(inputs, wq, bq, wk, bk, wv, bv, wo, bo):
    from concourse.bass_utils import run_bass_kernel_spmd

    nc = _get_nc()
    in_maps = _prep_core_inputs(inputs, wq, bq, wk, wv, wo)
    res = run_bass_kernel_spmd(nc, in_maps, list(range(N_CORES))).results

    wo_ = np.asarray(wo, np.float32)
    bias = (
        np.asarray(bo, np.float32)
        + wo_ @ np.tile(np.asarray(bv, np.float32), NUM_HEADS)
    )
    out = np.empty((B, S, D), np.float32)
    for b in range(B):
        acc = res[b * G]["part"].astype(np.float32).copy()
        for g in range(1, G):
            acc += res[b * G + g]["part"]
        out[b] = acc + bias
    return out


# revision 46
# speedup vs baseline: 1.3146x; 1.0080x over previous
"""MQA self-attention kernel for Trainium2, 8 NeuronCores.

Reference computation (fp32):
    q = x @ wq.T + bq        -> [B,S,1024] -> heads via (hidden num_heads) split
    k = x @ wk.T + bk        -> [B,S,64]  (single shared KV head)
    v = x @ wv.T + bv
    scores = q @ k.T / 8 ; attn = softmax(scores) ; h = attn @ v
    out = merge_heads(h) @ wo.T + bo

Sharding (8 cores, no collectives): core c handles batch b=c//4 and head
group g=c%4 (4 of the 16 q-heads).  The shared K/V head is replicated.
Each core returns the partial output h_g @ wo_g.T [S, D]; the host sums
the 4 head-group partials per batch and adds the bias terms.

Math notes:
 - bk provably cancels in softmax; bv is folded into the host-side output
   bias (softmax rows sum to 1); softmax runs without max subtraction
   (scores ~ N(0,1), exp stays within bf16/f32 range).

Device pipeline (all operands bf16, PSUM f32; the Activation engine's
exp throughput ~133us/core is the hard floor, so every other engine is
kept strictly below it):
 - xT/weights land as bf16 (halves the input DMA), projections produce
   QT [2 heads stacked per 128 partitions], KT (two partition-offset
   copies so odd/even heads both get offset-matched operands) and VT.
 - V' = [V|1] built by PE-transpose (half 0) / DMA-transpose (half 1).
 - scores_T[k, q] per head per 1024-query block; exp on ScalarE in
   [128,1024] blocks writing bf16.
 - PV runs dense: h_un[q, 65] += exp_T[:, qtile].T @ V' accumulated over
   key tiles in PSUM (2x fewer PE rows than the hT-layout alternative).
 - normalize on eviction via per-partition reciprocal multiply; pairs of
   heads share an SBUF buffer that one DMA-transpose per pair flips into
   hT layout for the output projection.
 - half-1 projections and out-projection chunks are emitted as deferred
   PE tasks, one per exp slot, so the Tensor engine queue always has
   work but never starves the exp pipeline.
"""

from collections import deque

import numpy as np

NUM_HEADS = 16
Dh = 64
B, S, D = 2, 2048, 1024
G = 4            # head groups (cores per batch)
HG = 4           # heads per group
QD = HG * Dh     # 256 local q dims
NK = D // 128    # 8 contraction tiles for projections
NSK = S // 128   # 16 key tiles
W = 512          # matmul moving width
HALF = 1024      # query block / projection column half
N_CORES = 8

_CACHE = {}
_DEBUG = False


def _build_nc():
    from contextlib import ExitStack

    import concourse.bass as bass
    import concourse.mybir as mybir
    import concourse.tile as tile
    from concourse import bacc
    from concourse.masks import make_identity

    F32 = mybir.dt.float32
    BF16 = mybir.dt.bfloat16
    EXP = mybir.ActivationFunctionType.Exp

    nc = bacc.Bacc("TRN2", target_bir_lowering=False, debug=False)

    xT = nc.declare_dram_parameter("xT", [D, S], BF16, isOutput=False)
    wqT = nc.declare_dram_parameter("wqT", [D, QD], BF16, isOutput=False)
    wkvT = nc.declare_dram_parameter("wkvT", [D, 128], BF16, isOutput=False)
    woT = nc.declare_dram_parameter("woT", [QD, D], BF16, isOutput=False)
    bqp = nc.declare_dram_parameter("bq", [QD, 1], F32, isOutput=False)
    part = nc.declare_dram_parameter("part", [S, D], BF16, isOutput=True)
    if _DEBUG:
        dbg = {
            "d_qt": nc.declare_dram_parameter("d_qt", [128, 2 * S], BF16, isOutput=True),
            "d_kt2": nc.declare_dram_parameter("d_kt2", [128, S], BF16, isOutput=True),
            "d_v1": nc.declare_dram_parameter("d_v1", [128, NSK * 65], BF16, isOutput=True),
            "d_ht": nc.declare_dram_parameter("d_ht", [128, 2 * S], BF16, isOutput=True),
        }

    with tile.TileContext(nc) as tc, ExitStack() as ctx:
        const = ctx.enter_context(tc.tile_pool(name="const", bufs=1))
        persist = ctx.enter_context(tc.tile_pool(name="persist", bufs=1))

        wq_sb = const.tile([128, NK * QD], BF16)    # ktile kt at cols [kt*QD:+QD]
        wkv_sb = const.tile([128, NK * 128], BF16)  # cols 0:64 = wkT, 64:128 = wvT
        wo_sb = const.tile([128, 2 * D], BF16)      # q-ktile p at cols [p*D:+D]
        bq_sb = const.tile([128, 2], F32)
        ident = const.tile([128, 128], BF16)

        qt_sb = persist.tile([128, 2 * S], BF16)    # pt p cols [p*S:+S]; rows 0:64 head 2p, 64:128 head 2p+1
        kt2_sb = persist.tile([128, S], BF16)       # KT duplicated rows 0:64 and 64:128
        vt_sb = persist.tile([128, S], BF16)        # VT in rows 64:128
        v1_sb = persist.tile([128, NSK * 65], BF16)  # V' tile sk at cols [sk*65:+65]
        ht_sb = persist.tile([128, 2 * S], BF16)    # hT, q-ktile p at cols [p*S:+S]

        make_identity(nc, ident[:])
        nc.vector.memset(v1_sb[:], 1.0)  # pre-fill the softmax-denominator columns

        # ---- DMAs + half-0 projections --------------------------------
        # x lands in 4 tiles of 2 ktiles (half 0, fine overlap with the
        # projection chains) + 2 tiles of 4 ktiles (half 1).  Issue queues
        # are spread over SP/ACT/DVE so the ~1.2us per-DMA issue cost
        # doesn't serialize the critical path.
        xp = ctx.enter_context(tc.tile_pool(name="xp", bufs=1))
        x0 = [xp.tile([128, 2 * HALF], BF16, name=f"x0_{j}") for j in range(4)]
        x1 = [xp.tile([128, 2 * HALF], BF16, name=f"x1_{j}") for j in range(4)]

        def dma_x(eng, tiles, j, hf):
            eng.dma_start(
                tiles[j][:].rearrange("p (k c) -> p k c", c=HALF),
                xT[j * 256:(j + 1) * 256, hf * HALF:(hf + 1) * HALF].rearrange(
                    "(k p) c -> p k c", p=128),
            )

        # weights issue from the ACT queue (idle until the first exp at
        # ~12us), x tiles from SP: neither queue's serialized ~1.2-2us
        # per-DMA issue cost then gates the critical path
        nc.scalar.dma_start(
            wkv_sb[:].rearrange("p (k c) -> p k c", c=128),
            wkvT[:, :].rearrange("(k p) c -> p k c", p=128),
        )
        nc.scalar.dma_start(
            wq_sb[:].rearrange("p (k c) -> p k c", c=QD),
            wqT[:, :].rearrange("(k p) c -> p k c", p=128),
        )
        for j in range(4):
            dma_x(nc.sync, x0, j, 0)
        for j in range(4):
            dma_x(nc.sync, x1, j, 1)
        for p in range(2):
            nc.scalar.dma_start(bq_sb[:, p:p + 1], bqp[p * 128:(p + 1) * 128, :])
        nc.scalar.dma_start(
            wo_sb[:].rearrange("p (a c) -> p a c", c=D),
            woT[:, :].rearrange("(a p) c -> p a c", p=128),
        )

        def xcol(kt, hf, n):
            tiles = x0 if hf == 0 else x1
            return tiles[kt // 2][:, (kt % 2) * HALF + n * W:(kt % 2) * HALF + (n + 1) * W]

        def proj_kt_step(ps, wsb_col, kt, hf, width):
            for n in range(HALF // W):
                nc.tensor.matmul(
                    ps[:, n * W:(n + 1) * W],
                    lhsT=wsb_col(kt),
                    rhs=xcol(kt, hf, n),
                    start=(kt == 0), stop=(kt == NK - 1),
                )

        wkv_col = lambda kt: wkv_sb[:, kt * 128:(kt + 1) * 128]
        wq0_col = lambda kt: wq_sb[:, kt * QD:kt * QD + 128]
        wq1_col = lambda kt: wq_sb[:, kt * QD + 128:kt * QD + 256]

        with (
            tc.tile_pool(name="p0", bufs=1, space="PSUM") as p0,
            tc.tile_pool(name="trps", bufs=2, space="PSUM") as trps,
        ):
            vk_ps = p0.tile([128, HALF], F32, name="vk")
            q0_ps = p0.tile([128, HALF], F32, name="q0")
            # vk + q0 gate the first exp; q1 (head 1) runs as deferred
            # fill-in so the scheduler can't wedge it between early scores
            for kt in range(NK):
                proj_kt_step(vk_ps, wkv_col, kt, 0, HALF)
                proj_kt_step(q0_ps, wq0_col, kt, 0, HALF)
            nc.scalar.copy(kt2_sb[0:64, 0:HALF], vk_ps[0:64, :])
            nc.vector.tensor_scalar_add(qt_sb[:, 0:HALF], q0_ps[:], bq_sb[:, 0:1])
            nc.vector.tensor_copy(vt_sb[64:128, 0:HALF], vk_ps[64:128, :])
            nc.gpsimd.tensor_copy(kt2_sb[64:128, 0:HALF], kt2_sb[0:64, 0:HALF])
            # V' half 0 by PE transpose (DMA engines are busy with x half 1)
            for sk in range(NSK // 2):
                tr = trps.tile([128, Dh], BF16, name="tr")
                nc.tensor.transpose(
                    tr[:], vt_sb[64:128, sk * 128:(sk + 1) * 128],
                    ident[64:128, 64:128],
                )
                nc.vector.tensor_copy(v1_sb[:, sk * 65:sk * 65 + 64], tr[:])

        # ---- Phase 2: attention, with deferred fill-in PE tasks -------
        expp = ctx.enter_context(tc.tile_pool(name="expp", bufs=8))
        hpp = ctx.enter_context(tc.tile_pool(name="hpp", bufs=2))
        smalls = ctx.enter_context(tc.tile_pool(name="smalls", bufs=4))
        osbp = ctx.enter_context(tc.tile_pool(name="osbp", bufs=3))
        # scores/h_un PSUM lives in its own stack so the final-block flush
        # can reclaim those banks for a deeper out-projection pipeline
        scph_stack = ExitStack()
        scps = scph_stack.enter_context(tc.tile_pool(name="scps", bufs=2, space="PSUM"))
        hups = scph_stack.enter_context(tc.tile_pool(name="hups", bufs=1, space="PSUM"))

        state = {}

        def mk_vk1(kts):
            def t():
                if "vk1" not in state:
                    pvk = pvk_stack.enter_context(
                        tc.tile_pool(name="pvk", bufs=1, space="PSUM"))
                    state["vk1"] = pvk.tile([128, HALF], F32, name="vkps")
                for kt in kts:
                    proj_kt_step(state["vk1"], wkv_col, kt, 1, HALF)
            return t

        def t_kv1_evict():
            vk1 = state.pop("vk1")
            nc.vector.tensor_copy(kt2_sb[0:64, HALF:S], vk1[0:64, :])
            nc.vector.tensor_copy(vt_sb[64:128, HALF:S], vk1[64:128, :])
            nc.gpsimd.tensor_copy(kt2_sb[64:128, HALF:S], kt2_sb[0:64, HALF:S])
            pvk_stack.close()

        def t_v1_h1():
            # V' half 1 by PE transpose, in a short-lived 1-bank PSUM era
            # (dma_start_transpose mis-executes the 65-strided pattern on HW)
            with tc.tile_pool(name="trps1", bufs=2, space="PSUM") as trps1:
                for sk in range(NSK // 2, NSK):
                    tr = trps1.tile([128, Dh], BF16, name="tr1")
                    nc.tensor.transpose(
                        tr[:], vt_sb[64:128, sk * 128:(sk + 1) * 128],
                        ident[64:128, 64:128],
                    )
                    nc.vector.tensor_copy(v1_sb[:, sk * 65:sk * 65 + 64], tr[:])

        def mk_q1(which, col_fn, kts, hf):
            def t():
                if which in state:
                    ps = state[which]
                else:
                    if "pq" not in state:
                        state["pq"] = pq_stack.enter_context(
                            tc.tile_pool(name="pq", bufs=1, space="PSUM"))
                    ps = state[which] = state["pq"].tile([128, HALF], F32, name="qps")
                for kt in kts:
                    proj_kt_step(ps, col_fn, kt, hf, HALF)
            return t

        def mk_q1_evict(which, pt, hf):
            def t():
                ps = state.pop(which)
                c0 = pt * S + hf * HALF
                nc.vector.tensor_scalar_add(
                    qt_sb[:, c0:c0 + HALF], ps[:], bq_sb[:, pt:pt + 1]
                )
            return t

        def t_close_p1b():
            pq_stack.close()

        tasks = deque(
            [None, None,
             mk_vk1(range(0, 2)), mk_vk1(range(2, 4)),
             mk_vk1(range(4, 6)), mk_vk1(range(6, 8)),
             t_kv1_evict, t_v1_h1]
            + [mk_q1("q1h0", wq1_col, range(kt, kt + 2), 0) for kt in range(0, NK, 2)]
            + [mk_q1_evict("q1h0", 1, 0)]
            + [mk_q1("q0h1", wq0_col, range(kt, kt + 1), 1) for kt in range(NK)]
            + [mk_q1_evict("q0h1", 0, 1)]
            + [mk_q1("q1h1", wq1_col, range(kt, kt + 1), 1) for kt in range(NK)]
            + [mk_q1_evict("q1h1", 1, 1), t_close_p1b]
        )

        outps_stack = ExitStack()
        outps = None
        pvk_stack = ExitStack()
        pq_stack = ExitStack()

        def mk_outproj(sc_i, n, p):
            # block-0 out-projection fill-in: one matmul per task (~213ns)
            # so it never blows the per-exp-slot PE budget
            def t():
                nonlocal outps
                if outps is None:
                    outps = outps_stack.enter_context(
                        tc.tile_pool(name="outps", bufs=2, space="PSUM"))
                s = sc_i
                half = sc_i % 2
                if half == 0 and n == 0 and p == 0:
                    state["osb"] = osbp.tile([128, 2 * D], BF16, name="osb")
                o_sb = state["osb"]
                if p == 0:
                    state["ops"] = outps.tile([128, W], F32, name="ops")
                o_ps = state["ops"]
                nc.tensor.matmul(
                    o_ps[:],
                    lhsT=ht_sb[:, p * S + s * 128:p * S + (s + 1) * 128],
                    rhs=wo_sb[:, p * D + n * W:p * D + (n + 1) * W],
                    start=(p == 0), stop=(p == 1),
                )
                if p == 1:
                    nc.vector.tensor_copy(
                        o_sb[:, half * D + n * W:half * D + (n + 1) * W], o_ps[:])
                if half == 1 and n == 1 and p == 1:
                    nc.sync.dma_start(
                        part[(s - 1) * 128:(s + 1) * 128, :].rearrange(
                            "(c p) d -> p c d", p=128),
                        o_sb[:].rearrange("p (c d) -> p c d", d=D),
                    )
                    if s == (HALF // 128) - 1:
                        outps_stack.close()
            return t

        hp = None
        for bI in range(2):
            for h in range(HG):
                pt, hi = h // 2, h % 2
                qbase = pt * S + bI * HALF
                if hi == 0:
                    hp = hpp.tile([128, HALF], BF16, name="hp")
                hu = [hups.tile([128, 260], F32, name=f"hu{g}") for g in range(2)]
                ets = [None] * NSK

                def emit_pv(sk, qts=range(8), hu=hu, ets=ets):
                    for qt in qts:
                        nc.tensor.matmul(
                            hu[qt // 4][:, (qt % 4) * 65:(qt % 4) * 65 + 65],
                            lhsT=ets[sk][:, qt * 128:(qt + 1) * 128],
                            rhs=v1_sb[:, sk * 65:(sk + 1) * 65],
                            # start zeroes the whole 2KB PSUM bank (all 4 qt
                            # regions), so only the bank's first/last matmul
                            # opens/closes the accumulation group
                            start=(sk == 0 and qt % 4 == 0),
                            stop=(sk == NSK - 1 and qt % 4 == 3),
                        )

                for sk in range(NSK):
                    # PV trails exp by 4 slots: the first PV of a head waits
                    # on the previous head's norm-eviction freeing the hu
                    # banks, and a deeper pipeline keeps that wait off the
                    # in-order PE queue's critical path.
                    if sk >= 4:
                        emit_pv(sk - 4)
                    sc = scps.tile([128, HALF], F32, name="sc")
                    for n in range(2):
                        nc.tensor.matmul(
                            sc[:, n * W:(n + 1) * W],
                            lhsT=kt2_sb[hi * 64:(hi + 1) * 64, sk * 128:(sk + 1) * 128],
                            rhs=qt_sb[hi * 64:(hi + 1) * 64, qbase + n * W:qbase + (n + 1) * W],
                            start=True, stop=True,
                        )
                    et = expp.tile([128, HALF], BF16, name="et")
                    nc.scalar.activation(et[:], sc[:], EXP, scale=0.125)
                    ets[sk] = et
                    if tasks:
                        t = tasks.popleft()
                        if t is not None:
                            t()
                # drain each hu bank group separately so its norm-eviction
                # and hT transpose fire before the other group's PV tail
                rec = smalls.tile([128, 8], F32, name="rec")
                for g in range(2):
                    qts = range(g * 4, g * 4 + 4)
                    for sk in range(NSK - 4, NSK):
                        emit_pv(sk, qts)
                    for q4 in range(4):
                        nc.vector.reciprocal(
                            rec[:, g * 4 + q4:g * 4 + q4 + 1],
                            hu[g][:, q4 * 65 + 64:q4 * 65 + 65],
                        )
                    for qt in qts:
                        nc.vector.tensor_scalar_mul(
                            hp[:, qt * 128 + hi * 64:qt * 128 + hi * 64 + 64],
                            hu[g][:, (qt % 4) * 65:(qt % 4) * 65 + 64],
                            rec[:, qt:qt + 1],
                        )
                    if hi == 1:
                        lo = g * 4 * 128
                        dst = ht_sb[:, pt * S + bI * HALF + lo:
                                    pt * S + bI * HALF + lo + 4 * 128]
                        dst = dst.rearrange("p (a b) -> p a b", b=128)
                        nc.sync.dma_start_transpose(dst, hp[:, lo:lo + 4 * 128])
                if hi == 1 and pt == 1 and bI == 0:
                    for sc_i in range(HALF // 128):
                        for n in range(2):
                            for p in range(2):
                                tasks.append(mk_outproj(sc_i, n, p))
        while tasks:
            t = tasks.popleft()
            if t is not None:
                t()
        if outps is not None:
            outps_stack.close()

        # ---- final flush: block-1 out-projection with the reclaimed
        # scores/h_un banks giving a 4-deep PSUM pipeline ----------------
        scph_stack.close()
        with tc.tile_pool(name="outpsB", bufs=4, space="PSUM") as outpsB:
            for sc_i in range(HALF // 128):
                s = (HALF // 128) + sc_i
                half = sc_i % 2
                if half == 0:
                    state["osbB"] = osbp.tile([128, 2 * D], BF16, name="osbB")
                o_sb = state["osbB"]
                for n in range(2):
                    o_ps = outpsB.tile([128, W], F32, name="opsB")
                    for p in range(2):
                        nc.tensor.matmul(
                            o_ps[:],
                            lhsT=ht_sb[:, p * S + s * 128:p * S + (s + 1) * 128],
                            rhs=wo_sb[:, p * D + n * W:p * D + (n + 1) * W],
                            start=(p == 0), stop=(p == 1),
                        )
                    dst = o_sb[:, half * D + n * W:half * D + (n + 1) * W]
                    # 2:1 DVE:ACT eviction split keeps both engines under
                    # the ~854ns/chunk PE cadence
                    if n == 1 and sc_i % 3 != 2:
                        nc.scalar.copy(dst, o_ps[:])
                    else:
                        nc.vector.tensor_copy(dst, o_ps[:])
                if half == 1:
                    nc.sync.dma_start(
                        part[(s - 1) * 128:(s + 1) * 128, :].rearrange(
                            "(c p) d -> p c d", p=128),
                        o_sb[:].rearrange("p (c d) -> p c d", d=D),
                    )
        if _DEBUG:
            nc.sync.dma_start(dbg["d_qt"][:, :], qt_sb[:])
            nc.sync.dma_start(dbg["d_kt2"][:, :], kt2_sb[:])
            nc.sync.dma_start(dbg["d_v1"][:, :], v1_sb[:])
            nc.sync.dma_start(dbg["d_ht"][:, :], ht_sb[:])

    nc.finalize()
    return nc


def _get_nc():
    if "nc" not in _CACHE:
        _CACHE["nc"] = _build_nc()
    return _CACHE["nc"]


def _prep_core_inputs(inputs, wq, bq, wk, wv, wo):
    """Host-side shard prep: per-core transposed/rearranged bf16 operands."""
    from ml_dtypes import bfloat16

    xT = [np.ascontiguousarray(np.asarray(inputs[b], np.float32).T).astype(bfloat16)
          for b in range(B)]
    wq3 = np.asarray(wq, np.float32).reshape(Dh, NUM_HEADS, D)
    bq2 = np.asarray(bq, np.float32).reshape(Dh, NUM_HEADS)
    wkvT = np.ascontiguousarray(
        np.concatenate([np.asarray(wk, np.float32).T, np.asarray(wv, np.float32).T],
                       axis=1)
    ).astype(bfloat16)  # [1024, 128], K in cols 0:64
    wo_ = np.asarray(wo, np.float32)

    in_maps = []
    for c in range(N_CORES):
        b, g = divmod(c, G)
        heads = [g * HG + hl for hl in range(HG)]
        wqT_g = np.ascontiguousarray(
            np.concatenate([wq3[:, h, :].T for h in heads], axis=1)
        ).astype(bfloat16)
        bq_g = np.ascontiguousarray(
            np.concatenate([bq2[:, h] for h in heads]).reshape(QD, 1).astype(np.float32)
        )
        woT_g = np.ascontiguousarray(
            wo_[:, g * QD:(g + 1) * QD].T
        ).astype(bfloat16)  # [256, 1024]
        in_maps.append({
            "xT": xT[b],
            "wqT": wqT_g,
            "wkvT": wkvT,
            "woT": woT_g,
            "bq": bq_g,
        })
    return in_maps


def kernel(inputs, wq, bq, wk, bk, wv, bv, wo, bo):
    from concourse.bass_utils import run_bass_kernel_spmd

    nc = _get_nc()
    in_maps = _prep_core_inputs(inputs, wq, bq, wk, wv, wo)
    res = run_bass_kernel_spmd(nc, in_maps, list(range(N_CORES))).results

    wo_ = np.asarray(wo, np.float32)
    bias = (
        np.asarray(bo, np.float32)
        + wo_ @ np.tile(np.asarray(bv, np.float32), NUM_HEADS)
    )
    out = np.empty((B, S, D), np.float32)
    for b in range(B):
        acc = res[b * G]["part"].astype(np.float32).copy()
        for g in range(1, G):
            acc += res[b * G + g]["part"]
        out[b] = acc + bias
    return out
